# revision 1
# baseline (speedup 1.0000x reference)
"""Trainium2 Bass kernel for nn_AttentionSup (dense transformer attention block).

Computation (see reference):
  qkv = x @ W_qkv; per-head attention softmax(q k^T / sqrt(d)) v;
  domain-gate (tiny MLP + softmax over heads) multiplies the attention
  output per (batch, head, dim); out = gated @ W_out + b_out.

Sharding over 8 NeuronCores: (batch b in 0..3) x (head-group g in 0..1),
4 heads per core — data-parallel over batch, tensor-parallel over heads.
Each core computes a partial output [2048, 512] for its batch from its 4
heads; the host sums the two head-group partials per batch and adds b_out
(the "all-reduce after to_out", done on host since partials per batch live
on exactly 2 cores).

The tiny domain-gate MLP ([4x4] @ [4x32] @ [32x512] per batch) is computed
on the host and folded into the V projection weights (it scales O columns,
i.e. W_v columns). Softmax uses the unnormalized-exp + ones-column trick:
V_ext = [V | 1], so PV matmul also produces row sums; normalization is a
reciprocal broadcast multiply. exp skips max-subtraction (scores ~N(0,1),
max ~5 — no overflow risk in fp32).

All matmuls run in fp32r (single-pass reduced-precision fp32, ~4x faster
than fp32's two half-speed passes; measured end-to-end rel err 3.4e-4).
fp32 PSUM accumulation throughout.

Performance notes (from NTFF traces): the attention phase is paced by the
ScalarE exp stream (~1.3us per [128,1024] tile incl. semaphores). The
tensor engine must stay slightly busier than ScalarE or the PE HAM clock
gate drops it to 1.2 GHz for most of the phase (~2x matmul slowdown, worth
~80us): hence the software-pipelined PV (consumes the previous kt-pair's
exp), the interleaved final-projection matmuls, and the periodic bf16
keep-warm filler matmuls. Input DMAs are split across both HWDGE rings
and chunked per d-tile so the QKV matmuls start after the first ~1MB.
"""

import sys

sys.path.insert(0, "/opt/trn_rl_repo")

import numpy as np
from contextlib import ExitStack

import concourse.bass as bass
import concourse.tile as tile
from concourse import bacc, mybir
from concourse.bass_utils import run_bass_kernel_spmd


def _install_ntff_hook():
    """Provide antenv.axon_hooks (absent from the image) so
    run_bass_kernel_spmd(trace=True) can capture NTFF profiles under axon."""
    import types

    if "antenv.axon_hooks" in sys.modules:
        return
    mod = types.ModuleType("antenv.axon_hooks")
    mod._HOOK = None
    mod.set_axon_ntff_profile_hook = lambda h: setattr(mod, "_HOOK", h)
    mod.get_axon_ntff_profile_hook = lambda: mod._HOOK
    try:
        from trn_agent_boot.trn_boot import _ntff_profile_via_ctypes

        mod._HOOK = _ntff_profile_via_ctypes("/opt/axon/libaxon_pjrt.so")
    except Exception:
        pass
    sys.modules["antenv.axon_hooks"] = mod
    try:
        import antenv

        antenv.axon_hooks = mod
    except Exception:
        pass


_install_ntff_hook()

f32 = mybir.dt.float32
f32r = mybir.dt.float32r
Exp = mybir.ActivationFunctionType.Exp

# Problem shapes (hardcoded per contract)
B, N, D = 4, 2048, 512
HEADS, DH = 8, 64
INNER = HEADS * DH  # 512
SCALE = DH**-0.5
NCORES = 8
HG = 2  # head groups (tensor-parallel degree)
HPC = HEADS // HG  # 4 heads per core
F = HPC * DH  # 256 inner dims per core
NT = N // 128  # 16 n(token)-tiles
DT = D // 128  # 4 d-tiles
QC = 4  # q chunks of 512
KTP = NT // 2  # 8 kt-pairs

_NC_CACHE = {}


def _build():
    """Build + compile the per-core Bass program (same program on all cores)."""
    nc = bacc.Bacc("TRN2", target_bir_lowering=False, debug=False, num_devices=NCORES)

    xT_d = nc.dram_tensor("xT", [D, N], f32, kind="ExternalInput")
    wq_d = nc.dram_tensor("wq", [D, F], f32, kind="ExternalInput")
    wk_d = nc.dram_tensor("wk", [D, F], f32, kind="ExternalInput")
    wv_d = nc.dram_tensor("wv", [D, F], f32, kind="ExternalInput")  # gate-scaled
    wo_d = nc.dram_tensor("wo", [F, D], f32, kind="ExternalInput")
    ones_d = nc.dram_tensor("ones", [128, 64], f32, kind="ExternalInput")
    part_d = nc.dram_tensor("part", [N, D], f32, kind="ExternalOutput")

    with tile.TileContext(nc) as tc:
        with ExitStack() as ctx:
            persist = ctx.enter_context(tc.tile_pool(name="persist", bufs=1))

            # qT/kT: [f, n] layout, one tile per head-pair (f-tile).
            qt_sb = [
                persist.tile([128, N], f32r, tag=f"qt{i}", name=f"qt{i}")
                for i in range(2)
            ]
            kt_sb = [
                persist.tile([128, N], f32r, tag=f"kt{i}", name=f"kt{i}")
                for i in range(2)
            ]
            # V_ext natural layout: [ktok, nt, head, 64+1]
            v_sb = persist.tile([128, NT, HPC, 65], f32r, tag="v", name="v")
            # gated+normalized attention output O^T: [f, n], per head-pair
            og_sb = [
                persist.tile([128, N], f32r, tag=f"og{i}", name=f"og{i}")
                for i in range(2)
            ]
            wo_sb = persist.tile([128, 2, D], f32r, tag="wo", name="wo")
            wo_bf = persist.tile([128, 512], mybir.dt.bfloat16, tag="wobf", name="wobf")
            ones_sb = persist.tile([1, 64], f32r, tag="ones1", name="ones1")

            ones64_sb = persist.tile([128, 64], f32r, tag="ones64", name="ones64")
            warm_sb = persist.tile([1, 64], f32, tag="warm", name="warm")

            # ---------------- Phase 1: QKV projections ----------------
            # ph1 (xT + weight tiles) stays open through attention: the V
            # projection matmuls are interleaved into the attention stream.
            ph1 = ctx.enter_context(tc.tile_pool(name="ph1", bufs=1))
            with tc.tile_pool(name="ps1", bufs=8, space="PSUM") as ps1:
                # chunk the xT DMA per d-tile AND use one tile per chunk so
                # dependency tracking lets the first matmuls start after ~1MB
                # instead of waiting for the whole 4MB
                xt_sb = [
                    ph1.tile([128, N], f32r, tag=f"xt{dt}", name=f"xt{dt}")
                    for dt in range(DT)
                ]
                w_sb = {}
                for wname, w_d in (("wq", wq_d), ("wk", wk_d), ("wv", wv_d)):
                    w_sb[wname] = [
                        ph1.tile([128, F], f32r, tag=f"{wname}{dt}", name=f"{wname}{dt}")
                        for dt in range(DT)
                    ]
                xt_r = xT_d[:].rearrange("(dt p) n -> p dt n", p=128).bitcast(f32r)
                for dt in range(DT):
                    # weights ride the second HWDGE ring (qActDynamicHW) so
                    # they flow in parallel with the xT chunks on qSPDynamicHW
                    for wname, w_d in (("wq", wq_d), ("wk", wk_d), ("wv", wv_d)):
                        nc.scalar.dma_start(
                            w_sb[wname][dt][:],
                            w_d[:]
                            .rearrange("(dt p) f -> p dt f", p=128)
                            .bitcast(f32r)[:, dt],
                        )
                    nc.sync.dma_start(xt_sb[dt][:], xt_r[:, dt])
                    if dt == 0:
                        # small constants after the first xT chunk; wo on the
                        # weight (qAct) ring; warmup exp loads the ACT table
                        # (~2.7us) well before the first real exp
                        nc.sync.dma_start(ones_sb[:], ones_d[0:1, :].bitcast(f32r))
                        nc.sync.dma_start(ones64_sb[:], ones_d[:, :].bitcast(f32r))
                        nc.scalar.dma_start(
                            wo_sb[:],
                            wo_d[:]
                            .rearrange("(ft p) m -> p ft m", p=128)
                            .bitcast(f32r),
                        )
                        nc.scalar.activation(
                            warm_sb[:], ones_sb[:].bitcast(f32), Exp, scale=0.0
                        )
                        nc.vector.tensor_copy(wo_bf[:], wo_sb[:, 0, :].bitcast(f32))
                        nc.vector.tensor_copy(
                            v_sb[:].rearrange("p a b c -> p (a b) c")[:, :, 64],
                            ones64_sb[:],
                        )

                # qT/kT: [f, n] = W^T x^T ; lhsT = W[dtile, ftile], rhs =
                # xT[dtile, nchunk]. dt-outer over 8 live psum accumulators:
                # PE starts on the first xT chunk.
                def qk_proj(wname, dst):
                    tiles = [
                        ps1.tile([128, 512], f32, tag="mm", name=f"mm_ps{i}")
                        for i in range(8)
                    ]
                    for dt in range(DT):
                        for ft in range(2):
                            for qc in range(QC):
                                nc.tensor.matmul(
                                    tiles[ft * QC + qc][:],
                                    w_sb[wname][dt][:, ft * 128 : (ft + 1) * 128],
                                    xt_sb[dt][:, qc * 512 : (qc + 1) * 512],
                                    start=(dt == 0),
                                    stop=(dt == DT - 1),
                                )
                    for ft in range(2):
                        for qc in range(QC):
                            nc.vector.tensor_copy(
                                dst[ft][:, qc * 512 : (qc + 1) * 512],
                                tiles[ft * QC + qc][:],
                            )

                def v_proj_ps1(nt):
                    ps = ps1.tile([128, 512], f32, tag="mm", name="v0_ps")
                    for dt in range(DT):
                        nc.tensor.matmul(
                            ps[:, 0:F],
                            xt_sb[dt][:, nt * 128 : (nt + 1) * 128],
                            w_sb["wv"][dt][:],
                            start=(dt == 0),
                            stop=(dt == DT - 1),
                        )
                    nc.vector.tensor_copy(
                        v_sb[:, nt, :, 0:64],
                        ps[:, 0:F].rearrange("p (h e) -> p h e", e=64),
                    )

                qk_proj("wq", qt_sb)
                qk_proj("wk", kt_sb)
                for nt in range(NT):
                    v_proj_ps1(nt)


            # ---------------- Phase 2: attention ----------------
            # psS bufs=3 (6 banks) lets PE run ST matmuls ~2 kt-pairs ahead
            # of the exp on ACT, keeping the tensor engine dense enough to
            # hold the HAM clock at 2.4 GHz. psO bufs=2 (2 banks) pipelines
            # the per-(head, qchunk) accumulator across iterations.
            with (
                tc.tile_pool(name="ptp", bufs=6) as ptp,
                tc.tile_pool(name="normp", bufs=2) as normp,
                tc.tile_pool(name="psS", bufs=2, space="PSUM") as psS,
                tc.tile_pool(name="psO", bufs=2, space="PSUM") as psO,
                tc.tile_pool(name="psD", bufs=1, space="PSUM") as psD,
                tc.tile_pool(name="outp", bufs=4) as outp,
            ):

                def final_nt(nt):
                    ps = psD.tile([128, 512], f32, tag="F", name="f_ps")
                    for fhp in range(2):
                        nc.tensor.matmul(
                            ps[:],
                            og_sb[fhp][:, nt * 128 : (nt + 1) * 128],
                            wo_sb[:, fhp, :],
                            start=(fhp == 0),
                            stop=(fhp == 1),
                        )
                    ob = outp.tile([128, 512], f32, tag="ob", name="ob")
                    nc.vector.tensor_copy(ob[:], ps[:])
                    nc.sync.dma_start(part_d[nt * 128 : (nt + 1) * 128, :], ob[:])

                def pv_pair(o_ps, pt, hp, h01, ktp):
                    for j in range(2):
                        kt = 2 * ktp + j
                        nc.tensor.matmul(
                            o_ps[:],
                            v_sb[:, kt, hp * 2 + h01, :],
                            pt[:, j * 512 : (j + 1) * 512],
                            start=(kt == 0),
                            stop=(kt == NT - 1),
                        )

                def normalize(blk, o_ps):
                    # og = O[0:64] * (1 / sums), sums broadcast down 64
                    # partitions via a K=1 matmul with a ones lhsT
                    hp, qc, h01 = blk
                    off = h01 * 64
                    srow = normp.tile([1, 512], f32r, tag="srow", name="srow")
                    nc.vector.tensor_copy(srow[:], o_ps[64:65, :])
                    rs_ps = psD.tile([64, 512], f32, tag="D", name="rs_ps")
                    nc.tensor.matmul(
                        rs_ps[:], ones_sb[:], srow[:], start=True, stop=True
                    )
                    rinv = normp.tile([64, 512], f32, tag="rinv", name="rinv")
                    nc.vector.reciprocal_approx_fast(rinv[:], rs_ps[:])
                    nc.vector.tensor_tensor(
                        og_sb[hp][off : off + 64, qc * 512 : (qc + 1) * 512],
                        o_ps[0:64, :],
                        rinv[:],
                        mybir.AluOpType.mult,
                    )
                    if hp == 1:
                        # queue this q-chunk's final-projection n-tiles once
                        # both head-pairs' og columns exist; drained one per
                        # iteration to keep the PE stream smooth
                        if h01 == 1:
                            pending_finals.extend(range(qc * 4, qc * 4 + 4))

                # One flat software pipeline over all (head-pair, q-chunk,
                # head) blocks and kt-pairs: the PV matmuls consume the
                # PREVIOUS iteration's exp output (even across block
                # boundaries), so the tensor engine never waits on the
                # activation engine; keep-warm filler matmuls top PE pace up
                # to just above the exp pace so the HAM clock gate stays at
                # 2.4 GHz for the whole phase.
                pending_finals = []
                blocks = [
                    (hp, qc, h01)
                    for hp in range(2)
                    for qc in range(QC)
                    for h01 in range(2)
                ]
                prev = None  # (pt, o_ps, blk, ktp)
                o_cur = None
                it_count = 0
                for blk in blocks:
                    hp, qc, h01 = blk
                    off = h01 * 64
                    o_cur = psO.tile([65, 512], f32, tag="O", name="o_ps")
                    for ktp in range(KTP):
                        s_ps = psS.tile([128, 1024], f32, tag="S", name="s_ps")
                        for j in range(2):
                            kt = 2 * ktp + j
                            nc.tensor.matmul(
                                s_ps[:, j * 512 : (j + 1) * 512],
                                kt_sb[hp][off : off + 64, kt * 128 : (kt + 1) * 128],
                                qt_sb[hp][off : off + 64, qc * 512 : (qc + 1) * 512],
                                start=True,
                                stop=True,
                            )
                        n_dummy = 1 if it_count % 3 == 0 else 0
                        it_count += 1
                        if pending_finals:
                            final_nt(pending_finals.pop(0))
                            n_dummy = 0
                        d_ps = psD.tile([128, 512], f32, tag="D", name="d_ps")
                        for _ in range(n_dummy):
                            nc.tensor.matmul(
                                d_ps[:],
                                wo_bf[:, 0:128],
                                wo_bf[:],
                                start=True,
                                stop=True,
                            )
                        pt = ptp.tile([128, 1024], f32r, tag="PT", name="pt")
                        nc.scalar.activation(pt[:], s_ps[:], Exp, scale=SCALE)
                        if prev is not None:
                            p_pt, p_o, p_blk, p_ktp = prev
                            pv_pair(p_o, p_pt, p_blk[0], p_blk[2], p_ktp)
                            if p_ktp == KTP - 1:
                                normalize(p_blk, p_o)
                        prev = (pt, o_cur, blk, ktp)
                p_pt, p_o, p_blk, p_ktp = prev
                pv_pair(p_o, p_pt, p_blk[0], p_blk[2], p_ktp)
                normalize(p_blk, p_o)
                for nt in pending_finals:
                    final_nt(nt)


    nc.compile()
    return nc


def _get_nc():
    if "nc" not in _NC_CACHE:
        _NC_CACHE["nc"] = _build()
    return _NC_CACHE["nc"]


def _prepare_in_maps(x, domain_label, W_qkv, W_d1, b_d1, W_d2, b_d2, W_out, b_out):
    x = np.asarray(x, np.float32)
    domain_label = np.asarray(domain_label, np.float32)
    W_qkv = np.asarray(W_qkv, np.float32)
    W_d1 = np.asarray(W_d1, np.float32)
    b_d1 = np.asarray(b_d1, np.float32)
    W_d2 = np.asarray(W_d2, np.float32)
    b_d2 = np.asarray(b_d2, np.float32)
    W_out = np.asarray(W_out, np.float32)

    # host: domain gate MLP + softmax over heads (tiny)
    d1 = np.maximum(domain_label @ W_d1 + b_d1, 0.0)
    d = d1 @ W_d2 + b_d2  # [B, INNER]
    d = d.reshape(B, HEADS, DH)
    e = np.exp(d - d.max(axis=1, keepdims=True))
    gate = (e / e.sum(axis=1, keepdims=True)).reshape(B, INNER).astype(np.float32)

    ones = np.ones((128, 64), np.float32)
    in_maps = []
    for c in range(NCORES):
        b, g = c // HG, c % HG
        sl = slice(g * F, (g + 1) * F)
        in_maps.append(
            {
                "xT": np.ascontiguousarray(x[b].T),
                "wq": np.ascontiguousarray(W_qkv[:, sl]),
                "wk": np.ascontiguousarray(W_qkv[:, INNER:][:, sl]),
                "wv": np.ascontiguousarray(
                    W_qkv[:, 2 * INNER :][:, sl] * gate[b, sl][None, :]
                ),
                "wo": np.ascontiguousarray(W_out[sl, :]),
                "ones": ones,
            }
        )
    return in_maps


def _run(in_maps, trace=False, tmpdir=None):
    nc = _get_nc()
    return run_bass_kernel_spmd(
        nc, in_maps, list(range(NCORES)), trace=trace, tmpdir=tmpdir
    )


def _assemble(results, b_out):
    b_out = np.asarray(b_out, np.float32)
    out = np.empty((B, N, D), np.float32)
    for b in range(B):
        out[b] = results[HG * b]["part"] + results[HG * b + 1]["part"] + b_out
    return out


def kernel(x, domain_label, W_qkv, W_d1, b_d1, W_d2, b_d2, W_out, b_out):
    in_maps = _prepare_in_maps(
        x, domain_label, W_qkv, W_d1, b_d1, W_d2, b_d2, W_out, b_out
    )
    res = _run(in_maps, trace=False)
    return _assemble(res.results, b_out)



# revision 11
# speedup vs baseline: 1.0484x; 1.0484x over previous
"""Trainium2 Bass kernel for nn_AttentionSup (dense transformer attention block).

Computation (see reference):
  qkv = x @ W_qkv; per-head attention softmax(q k^T / sqrt(d)) v;
  domain-gate (tiny MLP + softmax over heads) multiplies the attention
  output per (batch, head, dim); out = gated @ W_out + b_out.

Sharding over 8 NeuronCores: (batch b in 0..3) x (head-group g in 0..1),
4 heads per core — data-parallel over batch, tensor-parallel over heads.
Each core computes a partial output [2048, 512] for its batch from its 4
heads; the host sums the two head-group partials per batch and adds b_out.

The tiny domain-gate MLP is computed on the host and folded into the V
projection weights. Softmax uses the unnormalized-exp + ones-column trick:
V_ext = [V | 1], so PV matmul also produces row sums.

Performance design (v2):
 - All big matmuls in bf16 (inputs converted on host): enables FWL fast
   weight load so the LDWEIGHTS no longer serializes with each matmul
   (~355ns -> ~225ns per 512-free matmul). f32 PSUM accumulation kept.
 - The exp stream (the old bottleneck: 128 x [128,1024] tiles, ~147us on
   ScalarE alone) is split across TWO engines: even iterations use the ACT
   exp table, odd iterations use a custom 8-stage DVE op (EXP2_BITS_ANT)
   that computes bits(2^t) in float arithmetic (Schraudolph magic-add +
   abs-quadratic mantissa correction, max rel err ~0.33%) and writes
   through the DVE's float->int32 output conversion; the PV matmul reads
   the int32 buffer bitcast back to f32r. Scores are pre-scaled by
   SCALE*log2(e)*2^23 (folded into W_q on host); ACT uses its free affine
   scale (ln2*2^-23) to eat the same prescale.
 - normalize srow copies and the final-projection PSUM->SBUF copies run on
   ScalarE (which has slack); reciprocal + gating multiply stay on DVE.
 - keep-warm dummy matmuls removed: the PE instruction stream is now the
   pacer (~85% busy), so the HAM clock stays at 2.4 GHz on its own.
"""

import sys

sys.path.insert(0, "/opt/trn_rl_repo")

import numpy as np
from contextlib import ExitStack

import concourse.bass as bass
import concourse.tile as tile
from concourse import bacc, mybir
from concourse.bass_utils import run_bass_kernel_spmd


def _install_ntff_hook():
    """Provide antenv.axon_hooks (absent from the image) so
    run_bass_kernel_spmd(trace=True) can capture NTFF profiles under axon."""
    import types

    if "antenv.axon_hooks" in sys.modules:
        return
    mod = types.ModuleType("antenv.axon_hooks")
    mod._HOOK = None
    mod.set_axon_ntff_profile_hook = lambda h: setattr(mod, "_HOOK", h)
    mod.get_axon_ntff_profile_hook = lambda: mod._HOOK
    try:
        from trn_agent_boot.trn_boot import _ntff_profile_via_ctypes

        mod._HOOK = _ntff_profile_via_ctypes("/opt/axon/libaxon_pjrt.so")
    except Exception:
        pass
    sys.modules["antenv.axon_hooks"] = mod
    try:
        import antenv

        antenv.axon_hooks = mod
    except Exception:
        pass


_install_ntff_hook()

f32 = mybir.dt.float32
f32r = mybir.dt.float32r
bf16 = mybir.dt.bfloat16
u16 = mybir.dt.uint16
Exp = mybir.ActivationFunctionType.Exp

# Problem shapes (hardcoded per contract)
B, N, D = 4, 2048, 512
HEADS, DH = 8, 64
INNER = HEADS * DH  # 512
SCALE = DH**-0.5
NCORES = 8
HG = 2  # head groups (tensor-parallel degree)
HPC = HEADS // HG  # 4 heads per core
F = HPC * DH  # 256 inner dims per core
NT = N // 128  # 16 n(token)-tiles
DT = D // 128  # 4 d-tiles
QC = 4  # q chunks of 512
KTP = NT // 2  # 8 kt-pairs

LOG2E = 1.4426950408889634
LN2 = 0.6931471805599453
# scores are computed pre-scaled by SCALE*log2e*2^7 (folded into W_q): the
# custom DVE exp op then produces uint16 = the bf16 BIT PATTERN of 2^t via
# the DVE's float->uint16 output conversion (top 16 bits of the would-be
# fp32 encoding, in 2^7 "bf16 mantissa" units).
M7 = 2.0**7
# abs-quadratic minimax fit of eps(f)=psi(f)-f (mantissa-domain correction)
EA, EB, EC = 0.33020161, -0.5103379, -0.08690382
MAGIC = 1.5 * 2.0**30

_NC_CACHE = {}
_DVE_OP_CACHE = {}


def _get_exp2_bits_op():
    """Register (once) and return the custom DVE op computing
    out_uint16 = bf16_bits(2^(in0 * 2^-7)) for in0 = t*2^7:
      m  = t + MAGIC         ; MAGIC=1.5*2^30: rounds t to i*2^7 (+MAGIC)
      w  = m - MAGIC         ; i*2^7 (exact)
      ad = |t - w|           ; |f|*2^7        (ABSOLUTE_DIFF)
      q  = ad + EB*2^7       ; (|f|+b)*2^7
      s  = q*q               ; (|f|+b)^2*2^14
      u  = s * (EA*2^-7)     ; a(|f|+b)^2*2^7
      r  = u + (127+EC)*2^7  ; (127+c+a(|f|+b)^2)*2^7   [via Src1 latch]
      v  = t + r             -> written as uint16 (value->uint conversion)
    The uint16 buffer bitcast to bf16 is 2^(t*2^-7) to ~0.7% rel err."""
    if "op" in _DVE_OP_CACHE:
        return _DVE_OP_CACHE["op"]
    from concourse import dve_ops
    from concourse.dve_spec import (
        Spec,
        Src0,
        C0,
        C1,
        C2,
        C3,
        Bin,
        AluOp,
        lower,
        _spill_c3_to_src1,
    )
    from concourse.dve_uop import DveOpSpec
    from concourse.bass import dve_ver_for

    name = "EXP2_BITS_ANT"
    t = Src0
    m = t + C0
    w = m - C0
    ad = Bin(AluOp.ABSOLUTE_DIFF, t, w)
    q = ad + C1
    s = q * q
    u = s * C2
    r = u + C3
    body = _spill_c3_to_src1(t + r)

    def _ref(in0, in1, s0, s1, imm2):
        t = np.asarray(in0, np.float32)
        m = (t + np.float32(s0)).astype(np.float32)
        w = (m - np.float32(s0)).astype(np.float32)
        ad = np.abs(t - w).astype(np.float32)
        qq = (ad + np.float32(s1)).astype(np.float32)
        ss = (qq * qq).astype(np.float32)
        u = (ss * np.float32(imm2)).astype(np.float32)
        r = (u + np.float32(in1.flat[0])).astype(np.float32)
        return (t + r).astype(np.float32)

    spec = Spec(body=body, reference=_ref)
    ver = dve_ver_for("TRN2")
    row = max(dve_ops._SUB_OPCODE_FOR_NAME.values()) + 1
    uops = lower(spec, ver=ver)
    sha = DveOpSpec(name=name, opcode=row, uops=uops, rd1_en=True).sha(ver)
    op = dve_ops.DveOp(name, spec, subdim=False, uops_sha={ver: sha})
    if name not in dve_ops._SUB_OPCODE_FOR_NAME:
        dve_ops._SUB_OPCODE_FOR_NAME[name] = row
        dve_ops.OPS.append(op)
        dve_ops.CUSTOM_DVE_SPECS[name] = spec
    _DVE_OP_CACHE["op"] = op
    return op


def _build():
    """Build + compile the per-core Bass program (same program on all cores)."""
    exp2_op = _get_exp2_bits_op()
    nc = bacc.Bacc("TRN2", target_bir_lowering=False, debug=False, num_devices=NCORES)

    xT_d = nc.dram_tensor("xT", [D, N], bf16, kind="ExternalInput")
    wq_d = nc.dram_tensor("wq", [D, F], bf16, kind="ExternalInput")  # prescaled
    wk_d = nc.dram_tensor("wk", [D, F], bf16, kind="ExternalInput")
    wv_d = nc.dram_tensor("wv", [D, F], bf16, kind="ExternalInput")  # gate-scaled
    wo_d = nc.dram_tensor("wo", [F, D], bf16, kind="ExternalInput")
    ones_d = nc.dram_tensor("ones", [128, 64], f32, kind="ExternalInput")
    part_d = nc.dram_tensor("part", [N, D], f32, kind="ExternalOutput")

    with tile.TileContext(nc) as tc:
        with ExitStack() as ctx:
            persist = ctx.enter_context(tc.tile_pool(name="persist", bufs=1))

            # qT/kT: [f, n] layout, one tile per head-pair (f-tile), bf16.
            qt_sb = [
                persist.tile([128, N], bf16, tag=f"qt{i}", name=f"qt{i}")
                for i in range(2)
            ]
            kt_sb = [
                persist.tile([128, N], bf16, tag=f"kt{i}", name=f"kt{i}")
                for i in range(2)
            ]
            # V_ext natural layout: [ktok, nt, head, 64+1], bf16
            v_sb = persist.tile([128, NT, HPC, 65], bf16, tag="v", name="v")
            # gated+normalized attention output O^T: [f, n], per head-pair, bf16
            og_sb = [
                persist.tile([128, N], bf16, tag=f"og{i}", name=f"og{i}")
                for i in range(2)
            ]
            wo_sb = persist.tile([128, 2, D], bf16, tag="wo", name="wo")
            ones_sb = persist.tile([1, 64], f32r, tag="ones1", name="ones1")

            ones64_sb = persist.tile([128, 64], f32r, tag="ones64", name="ones64")
            warm_sb = persist.tile([1, 64], f32, tag="warm", name="warm")
            kc_sb = persist.tile([128, 1], f32, tag="kc", name="kc")

            # ---------------- Phase 1: QKV projections ----------------
            ph1 = ctx.enter_context(tc.tile_pool(name="ph1", bufs=1))
            with tc.tile_pool(name="ps1", bufs=8, space="PSUM") as ps1:
                # chunk the xT DMA per d-tile so the first matmuls start
                # after ~0.5MB instead of the whole 2MB
                xt_sb = [
                    ph1.tile([128, N], bf16, tag=f"xt{dt}", name=f"xt{dt}")
                    for dt in range(DT)
                ]
                w_sb = {}
                for wname, w_d in (("wq", wq_d), ("wk", wk_d), ("wv", wv_d)):
                    w_sb[wname] = [
                        ph1.tile([128, F], bf16, tag=f"{wname}{dt}", name=f"{wname}{dt}")
                        for dt in range(DT)
                    ]
                xt_r = xT_d[:].rearrange("(dt p) n -> p dt n", p=128)
                for dt in range(DT):
                    # weights ride the second HWDGE ring (qActDynamicHW) so
                    # they flow in parallel with the xT chunks on qSPDynamicHW
                    for wname, w_d in (("wq", wq_d), ("wk", wk_d), ("wv", wv_d)):
                        nc.scalar.dma_start(
                            w_sb[wname][dt][:],
                            w_d[:].rearrange("(dt p) f -> p dt f", p=128)[:, dt],
                        )
                    nc.sync.dma_start(xt_sb[dt][:], xt_r[:, dt])
                    if dt == 0:
                        nc.sync.dma_start(
                            ones_sb[:], ones_d[0:1, :].bitcast(f32r)
                        )
                        nc.sync.dma_start(ones64_sb[:], ones_d[:, :].bitcast(f32r))
                        nc.scalar.dma_start(
                            wo_sb[:],
                            wo_d[:].rearrange("(ft p) m -> p ft m", p=128),
                        )
                        # warmup exp loads the ACT table (~2.7us) well before
                        # the first real exp
                        nc.scalar.activation(
                            warm_sb[:], ones_sb[:].bitcast(f32), Exp, scale=0.0
                        )
                        nc.vector.memset(kc_sb[:], float((127.0 + EC) * M7))
                        nc.vector.tensor_copy(
                            v_sb[:].rearrange("p a b c -> p (a b) c")[:, :, 64],
                            ones64_sb[:],
                        )

                # qT/kT: [f, n] = W^T x^T ; lhsT = W[dtile, ftile], rhs =
                # xT[dtile, nchunk]. dt-outer over 8 live psum accumulators.
                def qk_proj(wname, dst):
                    tiles = [
                        ps1.tile([128, 512], f32, tag="mm", name=f"mm_ps{i}")
                        for i in range(8)
                    ]
                    for dt in range(DT):
                        for ft in range(2):
                            for qc in range(QC):
                                nc.tensor.matmul(
                                    tiles[ft * QC + qc][:],
                                    w_sb[wname][dt][:, ft * 128 : (ft + 1) * 128],
                                    xt_sb[dt][:, qc * 512 : (qc + 1) * 512],
                                    start=(dt == 0),
                                    stop=(dt == DT - 1),
                                )
                    for ft in range(2):
                        for qc in range(QC):
                            nc.vector.tensor_copy(
                                dst[ft][:, qc * 512 : (qc + 1) * 512],
                                tiles[ft * QC + qc][:],
                            )

                def v_proj_ps1(nt):
                    ps = ps1.tile([128, 512], f32, tag="mm", name="v0_ps")
                    for dt in range(DT):
                        nc.tensor.matmul(
                            ps[:, 0:F],
                            xt_sb[dt][:, nt * 128 : (nt + 1) * 128],
                            w_sb["wv"][dt][:],
                            start=(dt == 0),
                            stop=(dt == DT - 1),
                        )
                    nc.vector.tensor_copy(
                        v_sb[:, nt, :, 0:64],
                        ps[:, 0:F].rearrange("p (h e) -> p h e", e=64),
                    )

                qk_proj("wq", qt_sb)
                qk_proj("wk", kt_sb)
                for nt in range(NT):
                    v_proj_ps1(nt)

            # ---------------- Phase 2: attention ----------------
            with (
                tc.tile_pool(name="ptp", bufs=6) as ptp,
                tc.tile_pool(name="normp", bufs=2) as normp,
                tc.tile_pool(name="psS", bufs=2, space="PSUM") as psS,
                tc.tile_pool(name="psO", bufs=2, space="PSUM") as psO,
                tc.tile_pool(name="psD", bufs=1, space="PSUM") as psD,
                tc.tile_pool(name="outp", bufs=4) as outp,
            ):

                def final_nt(nt):
                    ps = psD.tile([128, 512], f32, tag="F", name="f_ps")
                    for fhp in range(2):
                        nc.tensor.matmul(
                            ps[:],
                            og_sb[fhp][:, nt * 128 : (nt + 1) * 128],
                            wo_sb[:, fhp, :],
                            start=(fhp == 0),
                            stop=(fhp == 1),
                        )
                    ob = outp.tile([128, 512], f32, tag="ob", name="ob")
                    nc.scalar.copy(ob[:], ps[:])
                    nc.sync.dma_start(part_d[nt * 128 : (nt + 1) * 128, :], ob[:])

                def pv_pair(o_ps, pt, hp, h01, ktp):
                    for j in range(2):
                        kt = 2 * ktp + j
                        nc.tensor.matmul(
                            o_ps[:],
                            v_sb[:, kt, hp * 2 + h01, :],
                            pt[:, j * 512 : (j + 1) * 512],
                            start=(kt == 0),
                            stop=(kt == NT - 1),
                        )

                def normalize(blk, o_ps):
                    # og = O[0:64] * (1 / sums), sums broadcast down 64
                    # partitions via a K=1 matmul with a ones lhsT
                    hp, qc, h01 = blk
                    off = h01 * 64
                    srow = normp.tile([1, 512], f32r, tag="srow", name="srow")
                    nc.scalar.copy(srow[:], o_ps[64:65, :])
                    rs_ps = psD.tile([64, 512], f32, tag="D", name="rs_ps")
                    nc.tensor.matmul(
                        rs_ps[:], ones_sb[:], srow[:], start=True, stop=True
                    )
                    rinv = normp.tile([64, 512], f32, tag="rinv", name="rinv")
                    nc.vector.reciprocal_approx_fast(rinv[:], rs_ps[:])
                    nc.vector.tensor_tensor(
                        og_sb[hp][off : off + 64, qc * 512 : (qc + 1) * 512],
                        o_ps[0:64, :],
                        rinv[:],
                        mybir.AluOpType.mult,
                    )
                    if hp == 1 and h01 == 1:
                        pending_finals.extend(range(qc * 4, qc * 4 + 4))

                # One flat software pipeline over all (head-pair, q-chunk,
                # head) blocks and kt-pairs; PV consumes the PREVIOUS
                # iteration's exp output. Exp alternates between the ACT
                # table engine (even iterations) and the custom DVE
                # bit-trick op (odd iterations), doubling exp throughput.
                pending_finals = []
                blocks = [
                    (hp, qc, h01)
                    for hp in range(2)
                    for qc in range(QC)
                    for h01 in range(2)
                ]
                prev = None  # (pt, o_ps, blk, ktp)
                it_count = 0
                for blk in blocks:
                    hp, qc, h01 = blk
                    off = h01 * 64
                    o_cur = psO.tile([65, 512], f32, tag="O", name="o_ps")
                    for ktp in range(KTP):
                        s_ps = psS.tile([128, 1024], f32, tag="S", name="s_ps")
                        for j in range(2):
                            kt = 2 * ktp + j
                            nc.tensor.matmul(
                                s_ps[:, j * 512 : (j + 1) * 512],
                                kt_sb[hp][off : off + 64, kt * 128 : (kt + 1) * 128],
                                qt_sb[hp][off : off + 64, qc * 512 : (qc + 1) * 512],
                                start=True,
                                stop=True,
                            )
                        if pending_finals:
                            final_nt(pending_finals.pop(0))
                        pt = ptp.tile([128, 1024], bf16, tag="PT", name="pt")
                        if it_count % 2 == 0:
                            nc.scalar.activation(
                                pt[:], s_ps[:], Exp, scale=LN2 / M7
                            )
                        else:
                            nc.vector._custom_dve(
                                exp2_op,
                                out=pt[:].bitcast(u16),
                                in0=s_ps[:],
                                in1=kc_sb[:],
                                s0=MAGIC,
                                s1=EB * M7,
                                imm2=EA / M7,
                            )
                        it_count += 1
                        if prev is not None:
                            p_pt, p_o, p_blk, p_ktp = prev
                            pv_pair(p_o, p_pt, p_blk[0], p_blk[2], p_ktp)
                            if p_ktp == KTP - 1:
                                normalize(p_blk, p_o)
                        prev = (pt, o_cur, blk, ktp)
                p_pt, p_o, p_blk, p_ktp = prev
                pv_pair(p_o, p_pt, p_blk[0], p_blk[2], p_ktp)
                normalize(p_blk, p_o)
                for nt in pending_finals:
                    final_nt(nt)

    nc.compile()
    return nc


def _get_nc():
    if "nc" not in _NC_CACHE:
        _NC_CACHE["nc"] = _build()
    return _NC_CACHE["nc"]


def _to_bf16(a):
    import ml_dtypes

    return np.ascontiguousarray(a).astype(ml_dtypes.bfloat16)


def _prepare_in_maps(x, domain_label, W_qkv, W_d1, b_d1, W_d2, b_d2, W_out, b_out):
    x = np.asarray(x, np.float32)
    domain_label = np.asarray(domain_label, np.float32)
    W_qkv = np.asarray(W_qkv, np.float32)
    W_d1 = np.asarray(W_d1, np.float32)
    b_d1 = np.asarray(b_d1, np.float32)
    W_d2 = np.asarray(W_d2, np.float32)
    b_d2 = np.asarray(b_d2, np.float32)
    W_out = np.asarray(W_out, np.float32)

    # host: domain gate MLP + softmax over heads (tiny)
    d1 = np.maximum(domain_label @ W_d1 + b_d1, 0.0)
    d = d1 @ W_d2 + b_d2  # [B, INNER]
    d = d.reshape(B, HEADS, DH)
    e = np.exp(d - d.max(axis=1, keepdims=True))
    gate = (e / e.sum(axis=1, keepdims=True)).reshape(B, INNER).astype(np.float32)

    qscale = np.float32(SCALE * LOG2E * M7)
    ones = np.ones((128, 64), np.float32)
    in_maps = []
    for c in range(NCORES):
        b, g = c // HG, c % HG
        sl = slice(g * F, (g + 1) * F)
        in_maps.append(
            {
                "xT": _to_bf16(x[b].T),
                "wq": _to_bf16(W_qkv[:, sl] * qscale),
                "wk": _to_bf16(W_qkv[:, INNER:][:, sl]),
                "wv": _to_bf16(
                    W_qkv[:, 2 * INNER :][:, sl] * gate[b, sl][None, :]
                ),
                "wo": _to_bf16(W_out[sl, :]),
                "ones": ones,
            }
        )
    return in_maps


def _run(in_maps, trace=False, tmpdir=None):
    nc = _get_nc()
    return run_bass_kernel_spmd(
        nc, in_maps, list(range(NCORES)), trace=trace, tmpdir=tmpdir
    )


def _assemble(results, b_out):
    b_out = np.asarray(b_out, np.float32)
    out = np.empty((B, N, D), np.float32)
    for b in range(B):
        out[b] = results[HG * b]["part"] + results[HG * b + 1]["part"] + b_out
    return out


def kernel(x, domain_label, W_qkv, W_d1, b_d1, W_d2, b_d2, W_out, b_out):
    in_maps = _prepare_in_maps(
        x, domain_label, W_qkv, W_d1, b_d1, W_d2, b_d2, W_out, b_out
    )
    res = _run(in_maps, trace=False)
    return _assemble(res.results, b_out)


# revision 12
# speedup vs baseline: 1.1338x; 1.0814x over previous
"""Trainium2 Bass kernel for nn_AttentionSup (dense transformer attention block).

Computation (see reference):
  qkv = x @ W_qkv; per-head attention softmax(q k^T / sqrt(d)) v;
  domain-gate (tiny MLP + softmax over heads) multiplies the attention
  output per (batch, head, dim); out = gated @ W_out + b_out.

Sharding over 8 NeuronCores: (batch b in 0..3) x (head-group g in 0..1),
4 heads per core — data-parallel over batch, tensor-parallel over heads.
Each core computes a partial output [2048, 512] for its batch from its 4
heads; the host sums the two head-group partials per batch and adds b_out.

The tiny domain-gate MLP is computed on the host and folded into the V
projection weights. Softmax uses the unnormalized-exp + ones-column trick:
V_ext = [V | 1], so PV matmul also produces row sums.

Performance design (v2):
 - All big matmuls in bf16 (inputs converted on host): enables FWL fast
   weight load so the LDWEIGHTS no longer serializes with each matmul
   (~355ns -> ~225ns per 512-free matmul). f32 PSUM accumulation kept.
 - The exp stream (the old bottleneck: 128 x [128,1024] tiles, ~147us on
   ScalarE alone) is split across TWO engines: even iterations use the ACT
   exp table, odd iterations use a custom 8-stage DVE op (EXP2_BITS_ANT)
   that computes bits(2^t) in float arithmetic (Schraudolph magic-add +
   abs-quadratic mantissa correction, max rel err ~0.33%) and writes
   through the DVE's float->int32 output conversion; the PV matmul reads
   the int32 buffer bitcast back to f32r. Scores are pre-scaled by
   SCALE*log2(e)*2^23 (folded into W_q on host); ACT uses its free affine
   scale (ln2*2^-23) to eat the same prescale.
 - normalize srow copies and the final-projection PSUM->SBUF copies run on
   ScalarE (which has slack); reciprocal + gating multiply stay on DVE.
 - keep-warm dummy matmuls removed: the PE instruction stream is now the
   pacer (~85% busy), so the HAM clock stays at 2.4 GHz on its own.
"""

import sys

sys.path.insert(0, "/opt/trn_rl_repo")

import numpy as np
from contextlib import ExitStack

import concourse.bass as bass
import concourse.tile as tile
from concourse import bacc, mybir
from concourse.bass_utils import run_bass_kernel_spmd


def _install_ntff_hook():
    """Provide antenv.axon_hooks (absent from the image) so
    run_bass_kernel_spmd(trace=True) can capture NTFF profiles under axon."""
    import types

    if "antenv.axon_hooks" in sys.modules:
        return
    mod = types.ModuleType("antenv.axon_hooks")
    mod._HOOK = None
    mod.set_axon_ntff_profile_hook = lambda h: setattr(mod, "_HOOK", h)
    mod.get_axon_ntff_profile_hook = lambda: mod._HOOK
    try:
        from trn_agent_boot.trn_boot import _ntff_profile_via_ctypes

        mod._HOOK = _ntff_profile_via_ctypes("/opt/axon/libaxon_pjrt.so")
    except Exception:
        pass
    sys.modules["antenv.axon_hooks"] = mod
    try:
        import antenv

        antenv.axon_hooks = mod
    except Exception:
        pass


_install_ntff_hook()

f32 = mybir.dt.float32
f32r = mybir.dt.float32r
bf16 = mybir.dt.bfloat16
u16 = mybir.dt.uint16
Exp = mybir.ActivationFunctionType.Exp

# Problem shapes (hardcoded per contract)
B, N, D = 4, 2048, 512
HEADS, DH = 8, 64
INNER = HEADS * DH  # 512
SCALE = DH**-0.5
NCORES = 8
HG = 2  # head groups (tensor-parallel degree)
HPC = HEADS // HG  # 4 heads per core
F = HPC * DH  # 256 inner dims per core
NT = N // 128  # 16 n(token)-tiles
DT = D // 128  # 4 d-tiles
QC = 4  # q chunks of 512
KTP = NT // 2  # 8 kt-pairs

LOG2E = 1.4426950408889634
LN2 = 0.6931471805599453
# scores are computed pre-scaled by SCALE*log2e*2^7 (folded into W_q): the
# custom DVE exp op then produces uint16 = the bf16 BIT PATTERN of 2^t via
# the DVE's float->uint16 output conversion (top 16 bits of the would-be
# fp32 encoding, in 2^7 "bf16 mantissa" units).
M7 = 2.0**7
# abs-quadratic minimax fit of eps(f)=psi(f)-f (mantissa-domain correction)
EA, EB, EC = 0.33020161, -0.5103379, -0.08690382
MAGIC = 1.5 * 2.0**30

_NC_CACHE = {}
_DVE_OP_CACHE = {}


def _get_exp2_bits_op():
    """Register (once) and return the custom DVE op computing
    out_uint16 = bf16_bits(2^(in0 * 2^-7)) for in0 = t*2^7:
      m  = t + MAGIC         ; MAGIC=1.5*2^30: rounds t to i*2^7 (+MAGIC)
      w  = m - MAGIC         ; i*2^7 (exact)
      ad = |t - w|           ; |f|*2^7        (ABSOLUTE_DIFF)
      q  = ad + EB*2^7       ; (|f|+b)*2^7
      s  = q*q               ; (|f|+b)^2*2^14
      u  = s * (EA*2^-7)     ; a(|f|+b)^2*2^7
      r  = u + (127+EC)*2^7  ; (127+c+a(|f|+b)^2)*2^7   [via Src1 latch]
      v  = t + r             -> written as uint16 (value->uint conversion)
    The uint16 buffer bitcast to bf16 is 2^(t*2^-7) to ~0.7% rel err."""
    if "op" in _DVE_OP_CACHE:
        return _DVE_OP_CACHE["op"]
    from concourse import dve_ops
    from concourse.dve_spec import (
        Spec,
        Src0,
        C0,
        C1,
        C2,
        C3,
        Bin,
        AluOp,
        lower,
        _spill_c3_to_src1,
    )
    from concourse.dve_uop import DveOpSpec
    from concourse.bass import dve_ver_for

    name = "EXP2_BITS_ANT"
    t = Src0
    m = t + C0
    w = m - C0
    ad = Bin(AluOp.ABSOLUTE_DIFF, t, w)
    q = ad + C1
    s = q * q
    u = s * C2
    r = u + C3
    body = _spill_c3_to_src1(t + r)

    def _ref(in0, in1, s0, s1, imm2):
        t = np.asarray(in0, np.float32)
        m = (t + np.float32(s0)).astype(np.float32)
        w = (m - np.float32(s0)).astype(np.float32)
        ad = np.abs(t - w).astype(np.float32)
        qq = (ad + np.float32(s1)).astype(np.float32)
        ss = (qq * qq).astype(np.float32)
        u = (ss * np.float32(imm2)).astype(np.float32)
        r = (u + np.float32(in1.flat[0])).astype(np.float32)
        return (t + r).astype(np.float32)

    spec = Spec(body=body, reference=_ref)
    ver = dve_ver_for("TRN2")
    row = max(dve_ops._SUB_OPCODE_FOR_NAME.values()) + 1
    uops = lower(spec, ver=ver)
    sha = DveOpSpec(name=name, opcode=row, uops=uops, rd1_en=True).sha(ver)
    op = dve_ops.DveOp(name, spec, subdim=False, uops_sha={ver: sha})
    if name not in dve_ops._SUB_OPCODE_FOR_NAME:
        dve_ops._SUB_OPCODE_FOR_NAME[name] = row
        dve_ops.OPS.append(op)
        dve_ops.CUSTOM_DVE_SPECS[name] = spec
    _DVE_OP_CACHE["op"] = op
    return op


def _build():
    """Build + compile the per-core Bass program (same program on all cores)."""
    exp2_op = _get_exp2_bits_op()
    nc = bacc.Bacc("TRN2", target_bir_lowering=False, debug=False, num_devices=NCORES)

    xT_d = nc.dram_tensor("xT", [D, N], bf16, kind="ExternalInput")
    wq_d = nc.dram_tensor("wq", [D, F], bf16, kind="ExternalInput")  # prescaled
    wk_d = nc.dram_tensor("wk", [D, F], bf16, kind="ExternalInput")
    wv_d = nc.dram_tensor("wv", [D, F], bf16, kind="ExternalInput")  # gate-scaled
    wo_d = nc.dram_tensor("wo", [F, D], bf16, kind="ExternalInput")
    ones_d = nc.dram_tensor("ones", [128, 64], f32, kind="ExternalInput")
    part_d = nc.dram_tensor("part", [N, D], f32, kind="ExternalOutput")

    with tile.TileContext(nc) as tc:
        with ExitStack() as ctx:
            persist = ctx.enter_context(tc.tile_pool(name="persist", bufs=1))

            # qT/kT: [f, n] layout, one tile per head-pair (f-tile), bf16.
            qt_sb = [
                persist.tile([128, N], bf16, tag=f"qt{i}", name=f"qt{i}")
                for i in range(2)
            ]
            kt_sb = [
                persist.tile([128, N], bf16, tag=f"kt{i}", name=f"kt{i}")
                for i in range(2)
            ]
            # V_ext natural layout: [ktok, nt, head, 64+1], bf16
            v_sb = persist.tile([128, NT, HPC, 65], bf16, tag="v", name="v")
            # gated+normalized attention output O^T: [f, n], per head-pair, bf16
            og_sb = [
                persist.tile([128, N], bf16, tag=f"og{i}", name=f"og{i}")
                for i in range(2)
            ]
            wo_sb = persist.tile([128, 2, D], bf16, tag="wo", name="wo")
            ones_sb = persist.tile([1, 64], f32r, tag="ones1", name="ones1")

            ones64_sb = persist.tile([128, 64], f32r, tag="ones64", name="ones64")
            warm_sb = persist.tile([1, 64], f32, tag="warm", name="warm")
            kc_sb = persist.tile([128, 1], f32, tag="kc", name="kc")

            # ---------------- Phase 1: QKV projections ----------------
            ph1 = ctx.enter_context(tc.tile_pool(name="ph1", bufs=1))
            with tc.tile_pool(name="ps1", bufs=8, space="PSUM") as ps1:
                # chunk the xT DMA per d-tile so the first matmuls start
                # after ~0.5MB instead of the whole 2MB
                xt_sb = [
                    ph1.tile([128, N], bf16, tag=f"xt{dt}", name=f"xt{dt}")
                    for dt in range(DT)
                ]
                w_sb = {}
                for wname, w_d in (("wq", wq_d), ("wk", wk_d), ("wv", wv_d)):
                    w_sb[wname] = [
                        ph1.tile([128, F], bf16, tag=f"{wname}{dt}", name=f"{wname}{dt}")
                        for dt in range(DT)
                    ]
                xt_r = xT_d[:].rearrange("(dt p) n -> p dt n", p=128)
                for dt in range(DT):
                    # weights ride the second HWDGE ring (qActDynamicHW) so
                    # they flow in parallel with the xT chunks on qSPDynamicHW
                    for wname, w_d in (("wq", wq_d), ("wk", wk_d), ("wv", wv_d)):
                        nc.scalar.dma_start(
                            w_sb[wname][dt][:],
                            w_d[:].rearrange("(dt p) f -> p dt f", p=128)[:, dt],
                        )
                    nc.sync.dma_start(xt_sb[dt][:], xt_r[:, dt])
                    if dt == 0:
                        nc.sync.dma_start(
                            ones_sb[:], ones_d[0:1, :].bitcast(f32r)
                        )
                        nc.sync.dma_start(ones64_sb[:], ones_d[:, :].bitcast(f32r))
                        nc.scalar.dma_start(
                            wo_sb[:],
                            wo_d[:].rearrange("(ft p) m -> p ft m", p=128),
                        )
                        # warmup exp loads the ACT table (~2.7us) well before
                        # the first real exp
                        nc.scalar.activation(
                            warm_sb[:], ones_sb[:].bitcast(f32), Exp, scale=0.0
                        )
                        nc.vector.memset(kc_sb[:], float((127.0 + EC) * M7))
                        nc.vector.tensor_copy(
                            v_sb[:].rearrange("p a b c -> p (a b) c")[:, :, 64],
                            ones64_sb[:],
                        )

                # qT/kT: [f, n] = W^T x^T ; lhsT = W[dtile, ftile], rhs =
                # xT[dtile, nchunk]. dt-outer over 8 live psum accumulators.
                def qk_proj(wname, dst):
                    tiles = [
                        ps1.tile([128, 512], f32, tag="mm", name=f"mm_ps{i}")
                        for i in range(8)
                    ]
                    for dt in range(DT):
                        for ft in range(2):
                            for qc in range(QC):
                                nc.tensor.matmul(
                                    tiles[ft * QC + qc][:],
                                    w_sb[wname][dt][:, ft * 128 : (ft + 1) * 128],
                                    xt_sb[dt][:, qc * 512 : (qc + 1) * 512],
                                    start=(dt == 0),
                                    stop=(dt == DT - 1),
                                )
                    for ft in range(2):
                        for qc in range(QC):
                            nc.vector.tensor_copy(
                                dst[ft][:, qc * 512 : (qc + 1) * 512],
                                tiles[ft * QC + qc][:],
                            )

                def v_proj_ps1(nt):
                    ps = ps1.tile([128, 512], f32, tag="mm", name="v0_ps")
                    for dt in range(DT):
                        nc.tensor.matmul(
                            ps[:, 0:F],
                            xt_sb[dt][:, nt * 128 : (nt + 1) * 128],
                            w_sb["wv"][dt][:],
                            start=(dt == 0),
                            stop=(dt == DT - 1),
                        )
                    nc.vector.tensor_copy(
                        v_sb[:, nt, :, 0:64],
                        ps[:, 0:F].rearrange("p (h e) -> p h e", e=64),
                    )

                qk_proj("wq", qt_sb)
                qk_proj("wk", kt_sb)
                for nt in range(NT):
                    v_proj_ps1(nt)

            # ---------------- Phase 2: attention ----------------
            # Iteration = (head-pair hp, q-chunk qc, key-tile kt): BOTH heads
            # of the pair in lockstep. The two ST matmuls hit different PE
            # row-groups (rows 0-63 / 64-127 via base_partition) and run
            # CONCURRENTLY in the array (~1x512cyc wall for both). One
            # [128,1024] exp covers both heads; PV runs with a 2-iteration
            # skew so the exp latency never enters the PE issue chain.
            # PSUM: psS 2x2 + psO 3 + psD 1 = 8 banks.
            from collections import deque

            with (
                tc.tile_pool(name="ptp", bufs=6) as ptp,
                tc.tile_pool(name="normp", bufs=4) as normp,
                tc.tile_pool(name="psS", bufs=2, space="PSUM") as psS,
                tc.tile_pool(name="psO", bufs=3, space="PSUM") as psO,
                tc.tile_pool(name="psD", bufs=1, space="PSUM") as psD,
                tc.tile_pool(name="outp", bufs=4) as outp,
            ):

                def final_nt(nt):
                    ps = psD.tile([128, 512], f32, tag="D", name="f_ps")
                    for fhp in range(2):
                        nc.tensor.matmul(
                            ps[:],
                            og_sb[fhp][:, nt * 128 : (nt + 1) * 128],
                            wo_sb[:, fhp, :],
                            start=(fhp == 0),
                            stop=(fhp == 1),
                        )
                    ob = outp.tile([128, 512], f32, tag="ob", name="ob")
                    nc.scalar.copy(ob[:], ps[:])
                    nc.sync.dma_start(part_d[nt * 128 : (nt + 1) * 128, :], ob[:])

                def normalize(hp, qc, h01, o_ps):
                    # og = O[0:64] * (1 / sums), sums broadcast down 64
                    # partitions via a K=1 matmul with a ones lhsT
                    off = h01 * 64
                    srow = normp.tile([1, 512], f32r, tag="srow", name="srow")
                    nc.scalar.copy(srow[:], o_ps[64:65, :])
                    rs_ps = psD.tile([64, 512], f32, tag="D", name="rs_ps")
                    nc.tensor.matmul(
                        rs_ps[:], ones_sb[:], srow[:], start=True, stop=True
                    )
                    rinv = normp.tile([64, 512], f32, tag="rinv", name="rinv")
                    nc.vector.reciprocal_approx_fast(rinv[:], rs_ps[:])
                    nc.vector.tensor_tensor(
                        og_sb[hp][off : off + 64, qc * 512 : (qc + 1) * 512],
                        o_ps[0:64, :],
                        rinv[:],
                        mybir.AluOpType.mult,
                    )
                    if hp == 1 and h01 == 1:
                        pending_finals.extend(range(qc * 4, qc * 4 + 4))

                def drain_one(ent):
                    pt, oA, oB, hp, qc, kt = ent
                    for h01, o_ps in ((0, oA), (1, oB)):
                        nc.tensor.matmul(
                            o_ps[:],
                            v_sb[:, kt, hp * 2 + h01, :],
                            pt[:, h01 * 512 : (h01 + 1) * 512],
                            start=(kt == 0),
                            stop=(kt == NT - 1),
                        )
                    if kt == NT - 1:
                        normalize(hp, qc, 0, oA)
                        normalize(hp, qc, 1, oB)

                pending_finals = []
                pend = deque()
                it_count = 0
                for hp in range(2):
                    for qc in range(QC):
                        o_A = psO.tile([65, 512], f32, tag="O", name="o_A")
                        o_B = psO.tile([65, 512], f32, tag="O", name="o_B")
                        for kt in range(NT):
                            s_ps = psS.tile([128, 1024], f32, tag="S", name="s_ps")
                            for h01 in range(2):
                                off = h01 * 64
                                nc.tensor.matmul(
                                    s_ps[:, h01 * 512 : (h01 + 1) * 512],
                                    kt_sb[hp][
                                        off : off + 64, kt * 128 : (kt + 1) * 128
                                    ],
                                    qt_sb[hp][
                                        off : off + 64, qc * 512 : (qc + 1) * 512
                                    ],
                                    start=True,
                                    stop=True,
                                )
                            if pending_finals:
                                final_nt(pending_finals.pop(0))
                            pt = ptp.tile([128, 1024], bf16, tag="PT", name="pt")
                            if it_count % 2 == 0:
                                nc.scalar.activation(
                                    pt[:], s_ps[:], Exp, scale=LN2 / M7
                                )
                            else:
                                nc.vector._custom_dve(
                                    exp2_op,
                                    out=pt[:].bitcast(u16),
                                    in0=s_ps[:],
                                    in1=kc_sb[:],
                                    s0=MAGIC,
                                    s1=EB * M7,
                                    imm2=EA / M7,
                                )
                            it_count += 1
                            pend.append((pt, o_A, o_B, hp, qc, kt))
                            if len(pend) > 2:
                                drain_one(pend.popleft())
                while pend:
                    drain_one(pend.popleft())
                for nt in pending_finals:
                    final_nt(nt)

    nc.compile()
    return nc


def _get_nc():
    if "nc" not in _NC_CACHE:
        _NC_CACHE["nc"] = _build()
    return _NC_CACHE["nc"]


def _to_bf16(a):
    import ml_dtypes

    return np.ascontiguousarray(a).astype(ml_dtypes.bfloat16)


def _prepare_in_maps(x, domain_label, W_qkv, W_d1, b_d1, W_d2, b_d2, W_out, b_out):
    x = np.asarray(x, np.float32)
    domain_label = np.asarray(domain_label, np.float32)
    W_qkv = np.asarray(W_qkv, np.float32)
    W_d1 = np.asarray(W_d1, np.float32)
    b_d1 = np.asarray(b_d1, np.float32)
    W_d2 = np.asarray(W_d2, np.float32)
    b_d2 = np.asarray(b_d2, np.float32)
    W_out = np.asarray(W_out, np.float32)

    # host: domain gate MLP + softmax over heads (tiny)
    d1 = np.maximum(domain_label @ W_d1 + b_d1, 0.0)
    d = d1 @ W_d2 + b_d2  # [B, INNER]
    d = d.reshape(B, HEADS, DH)
    e = np.exp(d - d.max(axis=1, keepdims=True))
    gate = (e / e.sum(axis=1, keepdims=True)).reshape(B, INNER).astype(np.float32)

    qscale = np.float32(SCALE * LOG2E * M7)
    ones = np.ones((128, 64), np.float32)
    in_maps = []
    for c in range(NCORES):
        b, g = c // HG, c % HG
        sl = slice(g * F, (g + 1) * F)
        in_maps.append(
            {
                "xT": _to_bf16(x[b].T),
                "wq": _to_bf16(W_qkv[:, sl] * qscale),
                "wk": _to_bf16(W_qkv[:, INNER:][:, sl]),
                "wv": _to_bf16(
                    W_qkv[:, 2 * INNER :][:, sl] * gate[b, sl][None, :]
                ),
                "wo": _to_bf16(W_out[sl, :]),
                "ones": ones,
            }
        )
    return in_maps


def _run(in_maps, trace=False, tmpdir=None):
    nc = _get_nc()
    return run_bass_kernel_spmd(
        nc, in_maps, list(range(NCORES)), trace=trace, tmpdir=tmpdir
    )


def _assemble(results, b_out):
    b_out = np.asarray(b_out, np.float32)
    out = np.empty((B, N, D), np.float32)
    for b in range(B):
        out[b] = results[HG * b]["part"] + results[HG * b + 1]["part"] + b_out
    return out


def kernel(x, domain_label, W_qkv, W_d1, b_d1, W_d2, b_d2, W_out, b_out):
    in_maps = _prepare_in_maps(
        x, domain_label, W_qkv, W_d1, b_d1, W_d2, b_d2, W_out, b_out
    )
    res = _run(in_maps, trace=False)
    return _assemble(res.results, b_out)


# revision 17
# speedup vs baseline: 1.3227x; 1.1666x over previous
"""Trainium2 Bass kernel for nn_AttentionSup (dense transformer attention block).

Computation (see reference):
  qkv = x @ W_qkv; per-head attention softmax(q k^T / sqrt(d)) v;
  domain-gate (tiny MLP + softmax over heads) multiplies the attention
  output per (batch, head, dim); out = gated @ W_out + b_out.

Sharding over 8 NeuronCores: (batch b in 0..3) x (head-group g in 0..1),
4 heads per core — data-parallel over batch, tensor-parallel over heads.
Each core computes a partial output [2048, 512] for its batch from its 4
heads; the host sums the two head-group partials per batch and adds b_out.

The tiny domain-gate MLP is computed on the host and folded into the V
projection weights. Softmax uses the unnormalized-exp + ones-column trick:
V_ext = [V | 1], so PV matmul also produces row sums.

Performance design (v2):
 - All big matmuls in bf16 (inputs converted on host): enables FWL fast
   weight load so the LDWEIGHTS no longer serializes with each matmul
   (~355ns -> ~225ns per 512-free matmul). f32 PSUM accumulation kept.
 - The exp stream (the old bottleneck: 128 x [128,1024] tiles, ~147us on
   ScalarE alone) is split across TWO engines: even iterations use the ACT
   exp table, odd iterations use a custom 8-stage DVE op (EXP2_BITS_ANT)
   that computes bits(2^t) in float arithmetic (Schraudolph magic-add +
   abs-quadratic mantissa correction, max rel err ~0.33%) and writes
   through the DVE's float->int32 output conversion; the PV matmul reads
   the int32 buffer bitcast back to f32r. Scores are pre-scaled by
   SCALE*log2(e)*2^23 (folded into W_q on host); ACT uses its free affine
   scale (ln2*2^-23) to eat the same prescale.
 - normalize srow copies and the final-projection PSUM->SBUF copies run on
   ScalarE (which has slack); reciprocal + gating multiply stay on DVE.
 - keep-warm dummy matmuls removed: the PE instruction stream is now the
   pacer (~85% busy), so the HAM clock stays at 2.4 GHz on its own.
"""

import sys

sys.path.insert(0, "/opt/trn_rl_repo")

import numpy as np
from contextlib import ExitStack

import concourse.bass as bass
import concourse.tile as tile
from concourse import bacc, mybir
from concourse.bass_utils import run_bass_kernel_spmd


def _install_ntff_hook():
    """Provide antenv.axon_hooks (absent from the image) so
    run_bass_kernel_spmd(trace=True) can capture NTFF profiles under axon."""
    import types

    if "antenv.axon_hooks" in sys.modules:
        return
    mod = types.ModuleType("antenv.axon_hooks")
    mod._HOOK = None
    mod.set_axon_ntff_profile_hook = lambda h: setattr(mod, "_HOOK", h)
    mod.get_axon_ntff_profile_hook = lambda: mod._HOOK
    try:
        from trn_agent_boot.trn_boot import _ntff_profile_via_ctypes

        mod._HOOK = _ntff_profile_via_ctypes("/opt/axon/libaxon_pjrt.so")
    except Exception:
        pass
    sys.modules["antenv.axon_hooks"] = mod
    try:
        import antenv

        antenv.axon_hooks = mod
    except Exception:
        pass


_install_ntff_hook()

f32 = mybir.dt.float32
f32r = mybir.dt.float32r
bf16 = mybir.dt.bfloat16
u16 = mybir.dt.uint16
Exp = mybir.ActivationFunctionType.Exp

# Problem shapes (hardcoded per contract)
B, N, D = 4, 2048, 512
HEADS, DH = 8, 64
INNER = HEADS * DH  # 512
SCALE = DH**-0.5
NCORES = 8
HG = 2  # head groups (tensor-parallel degree)
HPC = HEADS // HG  # 4 heads per core
F = HPC * DH  # 256 inner dims per core
NT = N // 128  # 16 n(token)-tiles
DT = D // 128  # 4 d-tiles
QC = 4  # q chunks of 512
KTP = NT // 2  # 8 kt-pairs

LOG2E = 1.4426950408889634
LN2 = 0.6931471805599453
# scores are computed pre-scaled by SCALE*log2e*2^7 (folded into W_q): the
# custom DVE exp op then produces uint16 = the bf16 BIT PATTERN of 2^t via
# the DVE's float->uint16 output conversion (top 16 bits of the would-be
# fp32 encoding, in 2^7 "bf16 mantissa" units).
M7 = 2.0**7
# abs-quadratic minimax fit of eps(f)=psi(f)-f (mantissa-domain correction)
EA, EB, EC = 0.33020161, -0.5103379, -0.08690382
MAGIC = 1.5 * 2.0**30

_NC_CACHE = {}
_DVE_OP_CACHE = {}


def _get_exp2_bits_op():
    """Register (once) and return the custom DVE op computing
    out_uint16 = bf16_bits(2^(in0 * 2^-7)) for in0 = t*2^7:
      m  = t + MAGIC         ; MAGIC=1.5*2^30: rounds t to i*2^7 (+MAGIC)
      w  = m - MAGIC         ; i*2^7 (exact)
      ad = |t - w|           ; |f|*2^7        (ABSOLUTE_DIFF)
      q  = ad + EB*2^7       ; (|f|+b)*2^7
      s  = q*q               ; (|f|+b)^2*2^14
      u  = s * (EA*2^-7)     ; a(|f|+b)^2*2^7
      r  = u + (127+EC)*2^7  ; (127+c+a(|f|+b)^2)*2^7   [via Src1 latch]
      v  = t + r             -> written as uint16 (value->uint conversion)
    The uint16 buffer bitcast to bf16 is 2^(t*2^-7) to ~0.7% rel err."""
    if "op" in _DVE_OP_CACHE:
        return _DVE_OP_CACHE["op"]
    from concourse import dve_ops
    from concourse.dve_spec import (
        Spec,
        Src0,
        C0,
        C1,
        C2,
        C3,
        Bin,
        AluOp,
        lower,
        _spill_c3_to_src1,
    )
    from concourse.dve_uop import DveOpSpec
    from concourse.bass import dve_ver_for

    name = "EXP2_BITS_ANT"
    t = Src0
    m = t + C0
    w = m - C0
    ad = Bin(AluOp.ABSOLUTE_DIFF, t, w)
    q = ad + C1
    s = q * q
    u = s * C2
    r = u + C3
    body = _spill_c3_to_src1(t + r)

    def _ref(in0, in1, s0, s1, imm2):
        t = np.asarray(in0, np.float32)
        m = (t + np.float32(s0)).astype(np.float32)
        w = (m - np.float32(s0)).astype(np.float32)
        ad = np.abs(t - w).astype(np.float32)
        qq = (ad + np.float32(s1)).astype(np.float32)
        ss = (qq * qq).astype(np.float32)
        u = (ss * np.float32(imm2)).astype(np.float32)
        r = (u + np.float32(in1.flat[0])).astype(np.float32)
        return (t + r).astype(np.float32)

    spec = Spec(body=body, reference=_ref)
    ver = dve_ver_for("TRN2")
    row = max(dve_ops._SUB_OPCODE_FOR_NAME.values()) + 1
    uops = lower(spec, ver=ver)
    sha = DveOpSpec(name=name, opcode=row, uops=uops, rd1_en=True).sha(ver)
    op = dve_ops.DveOp(name, spec, subdim=False, uops_sha={ver: sha})
    if name not in dve_ops._SUB_OPCODE_FOR_NAME:
        dve_ops._SUB_OPCODE_FOR_NAME[name] = row
        dve_ops.OPS.append(op)
        dve_ops.CUSTOM_DVE_SPECS[name] = spec
    _DVE_OP_CACHE["op"] = op
    return op


def _build():
    """Build + compile the per-core Bass program (same program on all cores)."""
    exp2_op = _get_exp2_bits_op()
    nc = bacc.Bacc("TRN2", target_bir_lowering=False, debug=False, num_devices=NCORES)

    xT_d = nc.dram_tensor("xT", [D, N], bf16, kind="ExternalInput")
    wq_d = nc.dram_tensor("wq", [D, F], bf16, kind="ExternalInput")  # prescaled
    wk_d = nc.dram_tensor("wk", [D, F], bf16, kind="ExternalInput")
    wv_d = nc.dram_tensor("wv", [D, F], bf16, kind="ExternalInput")  # gate-scaled
    wo_d = nc.dram_tensor("wo", [F, D], bf16, kind="ExternalInput")
    ones_d = nc.dram_tensor("ones", [128, 64], f32, kind="ExternalInput")
    part_d = nc.dram_tensor("part", [N, D], f32, kind="ExternalOutput")

    with tile.TileContext(nc) as tc:
        with ExitStack() as ctx:
            persist = ctx.enter_context(tc.tile_pool(name="persist", bufs=1))

            # qT/kT: [f, n] layout, one tile per head-pair (f-tile), bf16.
            qt_sb = [
                persist.tile([128, N], bf16, tag=f"qt{i}", name=f"qt{i}")
                for i in range(2)
            ]
            kt_sb = [
                persist.tile([128, N], bf16, tag=f"kt{i}", name=f"kt{i}")
                for i in range(2)
            ]
            # V_ext natural layout: [ktok, nt, head, 64+1], bf16
            v_sb = persist.tile([128, NT, HPC, 65], bf16, tag="v", name="v")
            # gated+normalized attention output O^T: [f, n], per head-pair, bf16
            og_sb = [
                persist.tile([128, N], bf16, tag=f"og{i}", name=f"og{i}")
                for i in range(2)
            ]
            wo_sb = persist.tile([128, 2, D], bf16, tag="wo", name="wo")
            ones_sb = persist.tile([1, 64], f32r, tag="ones1", name="ones1")

            ones64_sb = persist.tile([128, 64], f32r, tag="ones64", name="ones64")
            warm_sb = persist.tile([1, 64], f32, tag="warm", name="warm")
            kc_sb = persist.tile([128, 1], f32, tag="kc", name="kc")

            # ---------------- Phase 1: QKV projections ----------------
            ph1 = ctx.enter_context(tc.tile_pool(name="ph1", bufs=1))
            with tc.tile_pool(name="ps1", bufs=8, space="PSUM") as ps1:
                # chunk the xT DMA per (d-tile, n-half) — 8 x 256KB — so the
                # first matmuls start after ~1.5us and the PE stays
                # continuously busy (HAM warms early)
                xt_sb = [
                    [
                        ph1.tile([128, N // 2], bf16, tag=f"xt{dt}_{h}", name=f"xt{dt}_{h}")
                        for h in range(2)
                    ]
                    for dt in range(DT)
                ]
                w_sb = {}
                for wname, w_d in (("wq", wq_d), ("wk", wk_d), ("wv", wv_d)):
                    w_sb[wname] = [
                        ph1.tile([128, F], bf16, tag=f"{wname}{dt}", name=f"{wname}{dt}")
                        for dt in range(DT)
                    ]
                xt_r = xT_d[:].rearrange("(dt p) (h n) -> p dt h n", p=128, h=2)
                for dt in range(DT):
                    # weights ride the second HWDGE ring (qActDynamicHW) so
                    # they flow in parallel with the xT chunks on qSPDynamicHW
                    for wname, w_d in (("wq", wq_d), ("wk", wk_d), ("wv", wv_d)):
                        nc.scalar.dma_start(
                            w_sb[wname][dt][:],
                            w_d[:].rearrange("(dt p) f -> p dt f", p=128)[:, dt],
                        )
                    for h in range(2):
                        nc.sync.dma_start(xt_sb[dt][h][:], xt_r[:, dt, h])
                    if dt == 0:
                        nc.sync.dma_start(
                            ones_sb[:], ones_d[0:1, :].bitcast(f32r)
                        )
                        nc.sync.dma_start(ones64_sb[:], ones_d[:, :].bitcast(f32r))
                        nc.scalar.dma_start(
                            wo_sb[:],
                            wo_d[:].rearrange("(ft p) m -> p ft m", p=128),
                        )
                        # warmup exp loads the ACT table (~2.7us) well before
                        # the first real exp
                        nc.scalar.activation(
                            warm_sb[:], ones_sb[:].bitcast(f32), Exp, scale=0.0
                        )
                        nc.vector.memset(kc_sb[:], float((127.0 + EC) * M7))
                        nc.vector.tensor_copy(
                            v_sb[:].rearrange("p a b c -> p (a b) c")[:, :, 64],
                            ones64_sb[:],
                        )

                # qT/kT: [f, n] = W^T x^T ; lhsT = W[dtile, ftile], rhs =
                # xT[dtile, nchunk]. dt-outer over 8 live psum accumulators.
                def qk_proj(wname, dst):
                    tiles = [
                        ps1.tile([128, 512], f32, tag="mm", name=f"mm_ps{i}")
                        for i in range(8)
                    ]
                    for dt in range(DT):
                        for qc in range(QC):
                            for ft in range(2):
                                nc.tensor.matmul(
                                    tiles[ft * QC + qc][:],
                                    w_sb[wname][dt][:, ft * 128 : (ft + 1) * 128],
                                    xt_sb[dt][qc // 2][
                                        :, (qc % 2) * 512 : (qc % 2 + 1) * 512
                                    ],
                                    start=(dt == 0),
                                    stop=(dt == DT - 1),
                                )
                    for ft in range(2):
                        for qc in range(QC):
                            nc.vector.tensor_copy(
                                dst[ft][:, qc * 512 : (qc + 1) * 512],
                                tiles[ft * QC + qc][:],
                            )

                def v_proj_ps1(nt):
                    ps = ps1.tile([128, 512], f32, tag="mm", name="v0_ps")
                    for dt in range(DT):
                        nc.tensor.matmul(
                            ps[:, 0:F],
                            xt_sb[dt][nt // 8][
                                :, (nt % 8) * 128 : (nt % 8 + 1) * 128
                            ],
                            w_sb["wv"][dt][:],
                            start=(dt == 0),
                            stop=(dt == DT - 1),
                        )
                    nc.vector.tensor_copy(
                        v_sb[:, nt, :, 0:64],
                        ps[:, 0:F].rearrange("p (h e) -> p h e", e=64),
                    )

                qk_proj("wq", qt_sb)
                qk_proj("wk", kt_sb)
                for nt in range(NT):
                    v_proj_ps1(nt)

            # ---------------- Phase 2: attention ----------------
            # Iteration = (head-pair hp, q-chunk qc, key-tile kt): BOTH heads
            # of the pair in lockstep. The two ST matmuls hit different PE
            # row-groups (rows 0-63 / 64-127 via base_partition) and run
            # CONCURRENTLY in the array (~1x512cyc wall for both). One
            # [128,1024] exp covers both heads; PV runs with a 2-iteration
            # skew so the exp latency never enters the PE issue chain.
            # PSUM: psS 2x2 + psO 3 + psD 1 = 8 banks.
            from collections import deque

            with (
                tc.tile_pool(name="ptp", bufs=6) as ptp,
                tc.tile_pool(name="normp", bufs=4) as normp,
                tc.tile_pool(name="psS", bufs=2, space="PSUM") as psS,
                tc.tile_pool(name="psO", bufs=3, space="PSUM") as psO,
                tc.tile_pool(name="psD", bufs=1, space="PSUM") as psD,
                tc.tile_pool(name="outp", bufs=4) as outp,
            ):

                def final_nt(nt, on_act):
                    ps = psD.tile([128, 512], f32, tag="D", name="f_ps")
                    for fhp in range(2):
                        nc.tensor.matmul(
                            ps[:],
                            og_sb[fhp][:, nt * 128 : (nt + 1) * 128],
                            wo_sb[:, fhp, :],
                            start=(fhp == 0),
                            stop=(fhp == 1),
                        )
                    ob = outp.tile([128, 512], f32, tag="ob", name="ob")
                    # PSUM->SBUF bounce on whichever exp engine is idle this
                    # iteration, so the copy doesn't delay that engine's exp
                    if on_act:
                        nc.scalar.copy(ob[:], ps[:])
                    else:
                        nc.vector.tensor_copy(ob[:], ps[:])
                    nc.sync.dma_start(part_d[nt * 128 : (nt + 1) * 128, :], ob[:])

                def normalize(hp, qc, h01, o_ps):
                    # og = O[0:64] * (1 / sums), sums broadcast down 64
                    # partitions via a K=1 matmul with a ones lhsT
                    off = h01 * 64
                    srow = normp.tile([1, 512], f32r, tag="srow", name="srow")
                    if h01 == 0:
                        nc.scalar.copy(srow[:], o_ps[64:65, :])
                    else:
                        nc.vector.tensor_copy(srow[:], o_ps[64:65, :])
                    rs_ps = psD.tile([64, 512], f32, tag="D", name="rs_ps")
                    nc.tensor.matmul(
                        rs_ps[:], ones_sb[:], srow[:], start=True, stop=True
                    )
                    rinv = normp.tile([64, 512], f32, tag="rinv", name="rinv")
                    nc.vector.reciprocal_approx_fast(rinv[:], rs_ps[:])
                    nc.vector.tensor_tensor(
                        og_sb[hp][off : off + 64, qc * 512 : (qc + 1) * 512],
                        o_ps[0:64, :],
                        rinv[:],
                        mybir.AluOpType.mult,
                    )
                    if hp == 1 and h01 == 1:
                        pending_finals.extend(range(qc * 4, qc * 4 + 4))

                def drain_one(ent):
                    pt, oA, oB, hp, qc, kt = ent
                    for h01, o_ps in ((0, oA), (1, oB)):
                        nc.tensor.matmul(
                            o_ps[:],
                            v_sb[:, kt, hp * 2 + h01, :],
                            pt[:, h01 * 512 : (h01 + 1) * 512],
                            start=(kt == 0),
                            stop=(kt == NT - 1),
                        )
                    if kt == NT - 1:
                        normalize(hp, qc, 0, oA)
                        normalize(hp, qc, 1, oB)

                pending_finals = []
                pend = deque()
                it_count = 0
                for qc in range(QC):
                    for hp in range(2):
                        o_A = psO.tile([65, 512], f32, tag="O", name="o_A")
                        o_B = psO.tile([65, 512], f32, tag="O", name="o_B")
                        for kt in range(NT):
                            s_ps = psS.tile([128, 1024], f32, tag="S", name="s_ps")
                            for h01 in range(2):
                                off = h01 * 64
                                nc.tensor.matmul(
                                    s_ps[:, h01 * 512 : (h01 + 1) * 512],
                                    kt_sb[hp][
                                        off : off + 64, kt * 128 : (kt + 1) * 128
                                    ],
                                    qt_sb[hp][
                                        off : off + 64, qc * 512 : (qc + 1) * 512
                                    ],
                                    start=True,
                                    stop=True,
                                )
                            if pending_finals:
                                final_nt(pending_finals.pop(0), it_count % 2 == 1)
                            elif it_count < 6:
                                # keep-warm fillers through the pipeline-fill
                                # region so the HAM clock gate never sees a
                                # thin window at the QKV->attention seam
                                dps = psD.tile([128, 512], f32, tag="D", name="d_ps")
                                for _ in range(2):
                                    nc.tensor.matmul(
                                        dps[:],
                                        wo_sb[:, 0, 0:128],
                                        wo_sb[:, 0, :],
                                        start=True,
                                        stop=True,
                                    )
                            pt = ptp.tile([128, 1024], bf16, tag="PT", name="pt")
                            if it_count % 2 == 0:
                                nc.scalar.activation(
                                    pt[:], s_ps[:], Exp, scale=LN2 / M7
                                )
                            else:
                                nc.vector._custom_dve(
                                    exp2_op,
                                    out=pt[:].bitcast(u16),
                                    in0=s_ps[:],
                                    in1=kc_sb[:],
                                    s0=MAGIC,
                                    s1=EB * M7,
                                    imm2=EA / M7,
                                )
                            it_count += 1
                            pend.append((pt, o_A, o_B, hp, qc, kt))
                            if len(pend) > 3:
                                drain_one(pend.popleft())
                while pend:
                    drain_one(pend.popleft())
                for j, nt in enumerate(pending_finals):
                    final_nt(nt, j % 2 == 1)

    nc.compile()
    return nc


def _get_nc():
    if "nc" not in _NC_CACHE:
        _NC_CACHE["nc"] = _build()
    return _NC_CACHE["nc"]


def _to_bf16(a):
    import ml_dtypes

    return np.ascontiguousarray(a).astype(ml_dtypes.bfloat16)


def _prepare_in_maps(x, domain_label, W_qkv, W_d1, b_d1, W_d2, b_d2, W_out, b_out):
    x = np.asarray(x, np.float32)
    domain_label = np.asarray(domain_label, np.float32)
    W_qkv = np.asarray(W_qkv, np.float32)
    W_d1 = np.asarray(W_d1, np.float32)
    b_d1 = np.asarray(b_d1, np.float32)
    W_d2 = np.asarray(W_d2, np.float32)
    b_d2 = np.asarray(b_d2, np.float32)
    W_out = np.asarray(W_out, np.float32)

    # host: domain gate MLP + softmax over heads (tiny)
    d1 = np.maximum(domain_label @ W_d1 + b_d1, 0.0)
    d = d1 @ W_d2 + b_d2  # [B, INNER]
    d = d.reshape(B, HEADS, DH)
    e = np.exp(d - d.max(axis=1, keepdims=True))
    gate = (e / e.sum(axis=1, keepdims=True)).reshape(B, INNER).astype(np.float32)

    qscale = np.float32(SCALE * LOG2E * M7)
    ones = np.ones((128, 64), np.float32)
    in_maps = []
    for c in range(NCORES):
        b, g = c // HG, c % HG
        sl = slice(g * F, (g + 1) * F)
        in_maps.append(
            {
                "xT": _to_bf16(x[b].T),
                "wq": _to_bf16(W_qkv[:, sl] * qscale),
                "wk": _to_bf16(W_qkv[:, INNER:][:, sl]),
                "wv": _to_bf16(
                    W_qkv[:, 2 * INNER :][:, sl] * gate[b, sl][None, :]
                ),
                "wo": _to_bf16(W_out[sl, :]),
                "ones": ones,
            }
        )
    return in_maps


def _run(in_maps, trace=False, tmpdir=None):
    nc = _get_nc()
    return run_bass_kernel_spmd(
        nc, in_maps, list(range(NCORES)), trace=trace, tmpdir=tmpdir
    )


def _assemble(results, b_out):
    b_out = np.asarray(b_out, np.float32)
    out = np.empty((B, N, D), np.float32)
    for b in range(B):
        out[b] = results[HG * b]["part"] + results[HG * b + 1]["part"] + b_out
    return out


def kernel(x, domain_label, W_qkv, W_d1, b_d1, W_d2, b_d2, W_out, b_out):
    in_maps = _prepare_in_maps(
        x, domain_label, W_qkv, W_d1, b_d1, W_d2, b_d2, W_out, b_out
    )
    res = _run(in_maps, trace=False)
    return _assemble(res.results, b_out)


# revision 24
# speedup vs baseline: 1.3823x; 1.0451x over previous
"""Trainium2 Bass kernel for nn_AttentionSup (dense transformer attention block).

Computation (see reference):
  qkv = x @ W_qkv; per-head attention softmax(q k^T / sqrt(d)) v;
  domain-gate (tiny MLP + softmax over heads) multiplies the attention
  output per (batch, head, dim); out = gated @ W_out + b_out.

Sharding over 8 NeuronCores: (batch b in 0..3) x (head-group g in 0..1),
4 heads per core — data-parallel over batch, tensor-parallel over heads.
Each core computes a partial output [2048, 512] for its batch from its 4
heads; the host sums the two head-group partials per batch and adds b_out.

The tiny domain-gate MLP is computed on the host and folded into the V
projection weights. Softmax uses the unnormalized-exp + ones-column trick:
V_ext = [V | 1], so PV matmul also produces row sums.

Performance design (v2):
 - All big matmuls in bf16 (inputs converted on host): enables FWL fast
   weight load so the LDWEIGHTS no longer serializes with each matmul
   (~355ns -> ~225ns per 512-free matmul). f32 PSUM accumulation kept.
 - The exp stream (the old bottleneck: 128 x [128,1024] tiles, ~147us on
   ScalarE alone) is split across TWO engines: even iterations use the ACT
   exp table, odd iterations use a custom 8-stage DVE op (EXP2_BITS_ANT)
   that computes bits(2^t) in float arithmetic (Schraudolph magic-add +
   abs-quadratic mantissa correction, max rel err ~0.33%) and writes
   through the DVE's float->int32 output conversion; the PV matmul reads
   the int32 buffer bitcast back to f32r. Scores are pre-scaled by
   SCALE*log2(e)*2^23 (folded into W_q on host); ACT uses its free affine
   scale (ln2*2^-23) to eat the same prescale.
 - normalize srow copies and the final-projection PSUM->SBUF copies run on
   ScalarE (which has slack); reciprocal + gating multiply stay on DVE.
 - keep-warm dummy matmuls removed: the PE instruction stream is now the
   pacer (~85% busy), so the HAM clock stays at 2.4 GHz on its own.
"""

import sys

sys.path.insert(0, "/opt/trn_rl_repo")

import numpy as np
from contextlib import ExitStack

import concourse.bass as bass
import concourse.tile as tile
from concourse import bacc, mybir
from concourse.bass_utils import run_bass_kernel_spmd


def _install_ntff_hook():
    """Provide antenv.axon_hooks (absent from the image) so
    run_bass_kernel_spmd(trace=True) can capture NTFF profiles under axon."""
    import types

    if "antenv.axon_hooks" in sys.modules:
        return
    mod = types.ModuleType("antenv.axon_hooks")
    mod._HOOK = None
    mod.set_axon_ntff_profile_hook = lambda h: setattr(mod, "_HOOK", h)
    mod.get_axon_ntff_profile_hook = lambda: mod._HOOK
    try:
        from trn_agent_boot.trn_boot import _ntff_profile_via_ctypes

        mod._HOOK = _ntff_profile_via_ctypes("/opt/axon/libaxon_pjrt.so")
    except Exception:
        pass
    sys.modules["antenv.axon_hooks"] = mod
    try:
        import antenv

        antenv.axon_hooks = mod
    except Exception:
        pass


_install_ntff_hook()

f32 = mybir.dt.float32
f32r = mybir.dt.float32r
bf16 = mybir.dt.bfloat16
u16 = mybir.dt.uint16
Exp = mybir.ActivationFunctionType.Exp

# Problem shapes (hardcoded per contract)
B, N, D = 4, 2048, 512
HEADS, DH = 8, 64
INNER = HEADS * DH  # 512
SCALE = DH**-0.5
NCORES = 8
HG = 2  # head groups (tensor-parallel degree)
HPC = HEADS // HG  # 4 heads per core
F = HPC * DH  # 256 inner dims per core
NT = N // 128  # 16 n(token)-tiles
DT = D // 128  # 4 d-tiles
QC = 4  # q chunks of 512
KTP = NT // 2  # 8 kt-pairs

LOG2E = 1.4426950408889634
LN2 = 0.6931471805599453
# scores are computed pre-scaled by SCALE*log2e*2^7 (folded into W_q): the
# custom DVE exp op then produces uint16 = the bf16 BIT PATTERN of 2^t via
# the DVE's float->uint16 output conversion (top 16 bits of the would-be
# fp32 encoding, in 2^7 "bf16 mantissa" units).
M7 = 2.0**7
# abs-quadratic minimax fit of eps(f)=psi(f)-f (mantissa-domain correction)
EA, EB, EC = 0.33020161, -0.5103379, -0.08690382
MAGIC = 1.5 * 2.0**30

_NC_CACHE = {}
_DVE_OP_CACHE = {}


def _get_exp2_bits_op():
    """Register (once) and return the custom DVE op computing
    out_uint16 = bf16_bits(2^(in0 * 2^-7)) for in0 = t*2^7:
      m  = t + MAGIC         ; MAGIC=1.5*2^30: rounds t to i*2^7 (+MAGIC)
      w  = m - MAGIC         ; i*2^7 (exact)
      ad = |t - w|           ; |f|*2^7        (ABSOLUTE_DIFF)
      q  = ad + EB*2^7       ; (|f|+b)*2^7
      s  = q*q               ; (|f|+b)^2*2^14
      u  = s * (EA*2^-7)     ; a(|f|+b)^2*2^7
      r  = u + (127+EC)*2^7  ; (127+c+a(|f|+b)^2)*2^7   [via Src1 latch]
      v  = t + r             -> written as uint16 (value->uint conversion)
    The uint16 buffer bitcast to bf16 is 2^(t*2^-7) to ~0.7% rel err."""
    if "op" in _DVE_OP_CACHE:
        return _DVE_OP_CACHE["op"]
    from concourse import dve_ops
    from concourse.dve_spec import (
        Spec,
        Src0,
        C0,
        C1,
        C2,
        C3,
        Bin,
        AluOp,
        lower,
        _spill_c3_to_src1,
    )
    from concourse.dve_uop import DveOpSpec
    from concourse.bass import dve_ver_for

    name = "EXP2_BITS_ANT"
    t = Src0
    m = t + C0
    w = m - C0
    ad = Bin(AluOp.ABSOLUTE_DIFF, t, w)
    q = ad + C1
    s = q * q
    u = s * C2
    r = u + C3
    body = _spill_c3_to_src1(t + r)

    def _ref(in0, in1, s0, s1, imm2):
        t = np.asarray(in0, np.float32)
        m = (t + np.float32(s0)).astype(np.float32)
        w = (m - np.float32(s0)).astype(np.float32)
        ad = np.abs(t - w).astype(np.float32)
        qq = (ad + np.float32(s1)).astype(np.float32)
        ss = (qq * qq).astype(np.float32)
        u = (ss * np.float32(imm2)).astype(np.float32)
        r = (u + np.float32(in1.flat[0])).astype(np.float32)
        return (t + r).astype(np.float32)

    spec = Spec(body=body, reference=_ref)
    ver = dve_ver_for("TRN2")
    row = max(dve_ops._SUB_OPCODE_FOR_NAME.values()) + 1
    uops = lower(spec, ver=ver)
    sha = DveOpSpec(name=name, opcode=row, uops=uops, rd1_en=True).sha(ver)
    op = dve_ops.DveOp(name, spec, subdim=False, uops_sha={ver: sha})
    if name not in dve_ops._SUB_OPCODE_FOR_NAME:
        dve_ops._SUB_OPCODE_FOR_NAME[name] = row
        dve_ops.OPS.append(op)
        dve_ops.CUSTOM_DVE_SPECS[name] = spec
    _DVE_OP_CACHE["op"] = op
    return op


def _build():
    """Build + compile the per-core Bass program (same program on all cores)."""
    exp2_op = _get_exp2_bits_op()
    nc = bacc.Bacc("TRN2", target_bir_lowering=False, debug=False, num_devices=NCORES)

    xT_d = nc.dram_tensor("xT", [D, N], bf16, kind="ExternalInput")
    wq_d = nc.dram_tensor("wq", [D, F], bf16, kind="ExternalInput")  # prescaled
    wk_d = nc.dram_tensor("wk", [D, F], bf16, kind="ExternalInput")
    wv_d = nc.dram_tensor("wv", [D, F], bf16, kind="ExternalInput")  # gate-scaled
    wo_d = nc.dram_tensor("wo", [F, D], bf16, kind="ExternalInput")
    ones_d = nc.dram_tensor("ones", [128, 64], f32, kind="ExternalInput")
    part_d = nc.dram_tensor("part", [N, D], f32, kind="ExternalOutput")

    with tile.TileContext(nc) as tc:
        with ExitStack() as ctx:
            persist = ctx.enter_context(tc.tile_pool(name="persist", bufs=1))

            # qT/kT: [f, n] layout, one tile per head-pair (f-tile), bf16.
            qt_sb = [
                persist.tile([128, N], bf16, tag=f"qt{i}", name=f"qt{i}")
                for i in range(2)
            ]
            kt_sb = [
                persist.tile([128, N], bf16, tag=f"kt{i}", name=f"kt{i}")
                for i in range(2)
            ]
            # V_ext natural layout: [ktok, nt, head, 64+1], bf16
            v_sb = persist.tile([128, NT, HPC, 65], bf16, tag="v", name="v")
            # gated+normalized attention output O^T: [f, n], per head-pair, bf16
            og_sb = [
                persist.tile([128, N], bf16, tag=f"og{i}", name=f"og{i}")
                for i in range(2)
            ]
            wo_sb = persist.tile([128, 2, D], bf16, tag="wo", name="wo")
            ones_sb = persist.tile([1, 64], f32r, tag="ones1", name="ones1")

            ones64_sb = persist.tile([128, 64], f32r, tag="ones64", name="ones64")
            warm_sb = persist.tile([1, 64], f32, tag="warm", name="warm")
            kc_sb = persist.tile([128, 1], f32, tag="kc", name="kc")

            # ---------------- Phase 1: QKV projections ----------------
            ph1 = ctx.enter_context(tc.tile_pool(name="ph1", bufs=1))
            with tc.tile_pool(name="ps1", bufs=8, space="PSUM") as ps1:
                # chunk the xT DMA per (d-tile, n-half) — 8 x 256KB — so the
                # first matmuls start after ~1.5us and the PE stays
                # continuously busy (HAM warms early)
                xt_sb = [
                    [
                        ph1.tile([128, N // 2], bf16, tag=f"xt{dt}_{h}", name=f"xt{dt}_{h}")
                        for h in range(2)
                    ]
                    for dt in range(DT)
                ]
                w_sb = {}
                for wname, w_d in (("wq", wq_d), ("wk", wk_d), ("wv", wv_d)):
                    w_sb[wname] = [
                        ph1.tile([128, F], bf16, tag=f"{wname}{dt}", name=f"{wname}{dt}")
                        for dt in range(DT)
                    ]
                xt_r = xT_d[:].rearrange("(dt p) (h n) -> p dt h n", p=128, h=2)
                for dt in range(DT):
                    # weights ride the second HWDGE ring (qActDynamicHW) so
                    # they flow in parallel with the xT chunks on qSPDynamicHW
                    for wname, w_d in (("wq", wq_d), ("wk", wk_d), ("wv", wv_d)):
                        nc.scalar.dma_start(
                            w_sb[wname][dt][:],
                            w_d[:].rearrange("(dt p) f -> p dt f", p=128)[:, dt],
                        )
                    for h in range(2):
                        nc.sync.dma_start(xt_sb[dt][h][:], xt_r[:, dt, h])
                    if dt == 0:
                        nc.sync.dma_start(
                            ones_sb[:], ones_d[0:1, :].bitcast(f32r)
                        )
                        nc.sync.dma_start(ones64_sb[:], ones_d[:, :].bitcast(f32r))
                        nc.scalar.dma_start(
                            wo_sb[:],
                            wo_d[:].rearrange("(ft p) m -> p ft m", p=128),
                        )
                        # warmup exp loads the ACT table (~2.7us) well before
                        # the first real exp
                        nc.scalar.activation(
                            warm_sb[:], ones_sb[:].bitcast(f32), Exp, scale=0.0
                        )
                        nc.vector.memset(kc_sb[:], float((127.0 + EC) * M7))
                        nc.vector.tensor_copy(
                            v_sb[:].rearrange("p a b c -> p (a b) c")[:, :, 64],
                            ones64_sb[:],
                        )

                # qT/kT: [f, n] = W^T x^T ; lhsT = W[dtile, ftile], rhs =
                # xT[dtile, nchunk]. dt-outer over 8 live psum accumulators.
                def qk_proj_ft(wname, dst, ft):
                    # one (projection, f-tile) group: 16 matmuls into 4 psum
                    # accumulators, then 4 copies. Groups pipeline through the
                    # 8-buffer ps1 pool: group g's copies overlap group g+1's
                    # matmuls, so the PE never stalls on the bank recycle.
                    tiles = [
                        ps1.tile([128, 512], f32, tag="mm", name=f"mm_ps{i}")
                        for i in range(4)
                    ]
                    for dt in range(DT):
                        for qc in range(QC):
                            nc.tensor.matmul(
                                tiles[qc][:],
                                w_sb[wname][dt][:, ft * 128 : (ft + 1) * 128],
                                xt_sb[dt][qc // 2][
                                    :, (qc % 2) * 512 : (qc % 2 + 1) * 512
                                ],
                                start=(dt == 0),
                                stop=(dt == DT - 1),
                            )
                    for qc in range(QC):
                        nc.vector.tensor_copy(
                            dst[ft][:, qc * 512 : (qc + 1) * 512],
                            tiles[qc][:],
                        )

                def v_proj_ps1(nt):
                    ps = ps1.tile([128, 512], f32, tag="mm", name="v0_ps")
                    for dt in range(DT):
                        nc.tensor.matmul(
                            ps[:, 0:F],
                            xt_sb[dt][nt // 8][
                                :, (nt % 8) * 128 : (nt % 8 + 1) * 128
                            ],
                            w_sb["wv"][dt][:],
                            start=(dt == 0),
                            stop=(dt == DT - 1),
                        )
                    nc.vector.tensor_copy(
                        v_sb[:, nt, :, 0:64],
                        ps[:, 0:F].rearrange("p (h e) -> p h e", e=64),
                    )

                for wname, dst, ft in (
                    ("wq", qt_sb, 0),
                    ("wq", qt_sb, 1),
                    ("wk", kt_sb, 0),
                    ("wk", kt_sb, 1),
                ):
                    qk_proj_ft(wname, dst, ft)
                for nt in range(NT):
                    v_proj_ps1(nt)

            # ---------------- Phase 2: attention ----------------
            # Iteration = (head-pair hp, q-chunk qc, key-tile kt): BOTH heads
            # of the pair in lockstep. The two ST matmuls hit different PE
            # row-groups (rows 0-63 / 64-127 via base_partition) and run
            # CONCURRENTLY in the array (~1x512cyc wall for both). One
            # [128,1024] exp covers both heads; PV runs with a 2-iteration
            # skew so the exp latency never enters the PE issue chain.
            # PSUM: psS 2x2 + psO 3 + psD 1 = 8 banks.
            from collections import deque

            with (
                tc.tile_pool(name="ptp", bufs=6) as ptp,
                tc.tile_pool(name="normp", bufs=4) as normp,
                tc.tile_pool(name="psS", bufs=2, space="PSUM") as psS,
                tc.tile_pool(name="psO", bufs=3, space="PSUM") as psO,
                tc.tile_pool(name="psD", bufs=1, space="PSUM") as psD,
                tc.tile_pool(name="outp", bufs=4) as outp,
            ):

                def final_nt(nt, on_act, pool=None):
                    if pool is None:
                        ps = psD.tile([128, 512], f32, tag="D", name="f_ps")
                    else:
                        ps = pool.tile([128, 512], f32, tag="O", name="f_ps")
                    for fhp in range(2):
                        nc.tensor.matmul(
                            ps[:],
                            og_sb[fhp][:, nt * 128 : (nt + 1) * 128],
                            wo_sb[:, fhp, :],
                            start=(fhp == 0),
                            stop=(fhp == 1),
                        )
                    ob = outp.tile([128, 512], f32, tag="ob", name="ob")
                    # PSUM->SBUF bounce on whichever exp engine is idle this
                    # iteration, so the copy doesn't delay that engine's exp
                    if on_act:
                        nc.scalar.copy(ob[:], ps[:])
                    else:
                        nc.vector.tensor_copy(ob[:], ps[:])
                    # alternate output DMAs across both HWDGE rings
                    ring = nc.sync if nt % 2 == 0 else nc.scalar
                    ring.dma_start(part_d[nt * 128 : (nt + 1) * 128, :], ob[:])

                def normalize(hp, qc, h01, o_ps):
                    # og = O[0:64] * (1 / sums), sums broadcast down 64
                    # partitions via a K=1 matmul with a ones lhsT
                    off = h01 * 64
                    srow = normp.tile([1, 512], f32r, tag="srow", name="srow")
                    if h01 == 0:
                        nc.scalar.copy(srow[:], o_ps[64:65, :])
                    else:
                        nc.vector.tensor_copy(srow[:], o_ps[64:65, :])
                    rs_ps = psD.tile([64, 512], f32, tag="D", name="rs_ps")
                    nc.tensor.matmul(
                        rs_ps[:], ones_sb[:], srow[:], start=True, stop=True
                    )
                    rinv = normp.tile([64, 512], f32, tag="rinv", name="rinv")
                    nc.vector.reciprocal_approx_fast(rinv[:], rs_ps[:])
                    nc.vector.tensor_tensor(
                        og_sb[hp][off : off + 64, qc * 512 : (qc + 1) * 512],
                        o_ps[0:64, :],
                        rinv[:],
                        mybir.AluOpType.mult,
                    )
                    if hp == 1 and h01 == 1:
                        pending_finals.extend(range(qc * 4, qc * 4 + 4))

                def drain_one(ent):
                    # deferred-normalize stagger: B's normalize runs one
                    # drain later than A's so the block-end DVE burst
                    # (reciprocal + gating multiply) is half as tall and the
                    # exp stream hiccup stays within the 3-iteration pt skew
                    nonlocal deferred_norm
                    pt, oA, oB, hp, qc, kt = ent
                    for h01, o_ps in ((0, oA), (1, oB)):
                        nc.tensor.matmul(
                            o_ps[:],
                            v_sb[:, kt, hp * 2 + h01, :],
                            pt[:, h01 * 512 : (h01 + 1) * 512],
                            start=(kt == 0),
                            stop=(kt == NT - 1),
                        )
                    if deferred_norm is not None:
                        deferred_norm()
                        deferred_norm = None
                    if kt == NT - 1:
                        normalize(hp, qc, 0, oA)
                        deferred_norm = lambda: normalize(hp, qc, 1, oB)

                pending_finals = []
                pend = deque()
                deferred_norm = None
                it_count = 0
                for qc in range(QC):
                    for hp in range(2):
                        o_A = psO.tile([65, 512], f32, tag="O", name="o_A")
                        o_B = psO.tile([65, 512], f32, tag="O", name="o_B")
                        for kt in range(NT):
                            s_ps = psS.tile([128, 1024], f32, tag="S", name="s_ps")
                            for h01 in range(2):
                                off = h01 * 64
                                nc.tensor.matmul(
                                    s_ps[:, h01 * 512 : (h01 + 1) * 512],
                                    kt_sb[hp][
                                        off : off + 64, kt * 128 : (kt + 1) * 128
                                    ],
                                    qt_sb[hp][
                                        off : off + 64, qc * 512 : (qc + 1) * 512
                                    ],
                                    start=True,
                                    stop=True,
                                )
                            if pending_finals:
                                final_nt(pending_finals.pop(0), it_count % 2 == 1)
                            elif it_count < 6:
                                # keep-warm fillers through the pipeline-fill
                                # region so the HAM clock gate never sees a
                                # thin window at the QKV->attention seam
                                dps = psD.tile([128, 512], f32, tag="D", name="d_ps")
                                for _ in range(2):
                                    nc.tensor.matmul(
                                        dps[:],
                                        wo_sb[:, 0, 0:128],
                                        wo_sb[:, 0, :],
                                        start=True,
                                        stop=True,
                                    )
                            pt = ptp.tile([128, 1024], bf16, tag="PT", name="pt")
                            if it_count % 2 == 0:
                                nc.scalar.activation(
                                    pt[:], s_ps[:], Exp, scale=LN2 / M7
                                )
                            else:
                                nc.vector._custom_dve(
                                    exp2_op,
                                    out=pt[:].bitcast(u16),
                                    in0=s_ps[:],
                                    in1=kc_sb[:],
                                    s0=MAGIC,
                                    s1=EB * M7,
                                    imm2=EA / M7,
                                )
                            it_count += 1
                            pend.append((pt, o_A, o_B, hp, qc, kt))
                            if len(pend) > 3:
                                drain_one(pend.popleft())
                while pend:
                    drain_one(pend.popleft())
                if deferred_norm is not None:
                    deferred_norm()
                    deferred_norm = None
                # tail finals rotate through the (now free) 3-bank psO pool
                # so they pipeline instead of serializing on psD's one bank
                for j, nt in enumerate(pending_finals):
                    final_nt(nt, j % 2 == 1, pool=psO)

    nc.compile()
    return nc


def _get_nc():
    if "nc" not in _NC_CACHE:
        _NC_CACHE["nc"] = _build()
    return _NC_CACHE["nc"]


def _to_bf16(a):
    import ml_dtypes

    return np.ascontiguousarray(a).astype(ml_dtypes.bfloat16)


def _prepare_in_maps(x, domain_label, W_qkv, W_d1, b_d1, W_d2, b_d2, W_out, b_out):
    x = np.asarray(x, np.float32)
    domain_label = np.asarray(domain_label, np.float32)
    W_qkv = np.asarray(W_qkv, np.float32)
    W_d1 = np.asarray(W_d1, np.float32)
    b_d1 = np.asarray(b_d1, np.float32)
    W_d2 = np.asarray(W_d2, np.float32)
    b_d2 = np.asarray(b_d2, np.float32)
    W_out = np.asarray(W_out, np.float32)

    # host: domain gate MLP + softmax over heads (tiny)
    d1 = np.maximum(domain_label @ W_d1 + b_d1, 0.0)
    d = d1 @ W_d2 + b_d2  # [B, INNER]
    d = d.reshape(B, HEADS, DH)
    e = np.exp(d - d.max(axis=1, keepdims=True))
    gate = (e / e.sum(axis=1, keepdims=True)).reshape(B, INNER).astype(np.float32)

    qscale = np.float32(SCALE * LOG2E * M7)
    ones = np.ones((128, 64), np.float32)
    in_maps = []
    for c in range(NCORES):
        b, g = c // HG, c % HG
        sl = slice(g * F, (g + 1) * F)
        in_maps.append(
            {
                "xT": _to_bf16(x[b].T),
                "wq": _to_bf16(W_qkv[:, sl] * qscale),
                "wk": _to_bf16(W_qkv[:, INNER:][:, sl]),
                "wv": _to_bf16(
                    W_qkv[:, 2 * INNER :][:, sl] * gate[b, sl][None, :]
                ),
                "wo": _to_bf16(W_out[sl, :]),
                "ones": ones,
            }
        )
    return in_maps


def _run(in_maps, trace=False, tmpdir=None):
    nc = _get_nc()
    return run_bass_kernel_spmd(
        nc, in_maps, list(range(NCORES)), trace=trace, tmpdir=tmpdir
    )


def _assemble(results, b_out):
    b_out = np.asarray(b_out, np.float32)
    out = np.empty((B, N, D), np.float32)
    for b in range(B):
        out[b] = results[HG * b]["part"] + results[HG * b + 1]["part"] + b_out
    return out


def kernel(x, domain_label, W_qkv, W_d1, b_d1, W_d2, b_d2, W_out, b_out):
    in_maps = _prepare_in_maps(
        x, domain_label, W_qkv, W_d1, b_d1, W_d2, b_d2, W_out, b_out
    )
    res = _run(in_maps, trace=False)
    return _assemble(res.results, b_out)
